# revision 14
# baseline (speedup 1.0000x reference)
"""Trainium2 Bass kernel for nn_Net_16174846837292 (NNConv GNN message passing).

Strategy (graph-sharded, aggregation-folded, single fp16 a2 pass):
  pooled[g,o] = sum_{e: batch[dst[e]]=g} w_e * msg[e,o],  w_e = 1/max(cnt[dst_e],1)
  msg[e,o]    = sum_{k,i} e3[e,k]*h[src_e,i]*e4w[k,i*128+o] + sum_i h[src_e,i]*e4b[i*128+o]
  => pooled[g,o] = sum_k ZG_g[:,k]^T A2f[:,k*128+o] + HW_g^T Br
     ZG_g[i,k] = sum_{e in g} (w_e h[src_e,i]) e3[e,k],  HW_g[i] = sum_e w_e h[src_e,i]

Sharding: edges grouped by the graph of their destination node; 8 graphs per
core, so in-degree weights are per-edge host constants and NO collectives are
needed. Per-core edges pack into 8 slots of 192 (64-aligned segments).

Host precomputes w_e and pre-gathers x[src_e] per edge slot, so the device
kernel needs no histogram, no h DRAM round-trip, no indirect DMA, and no PE
transposes: the last layer of each MLP is computed edge-major by using the
previous layer's activations as the matmul stationary operand. All PE
operands are 16-bit (fp32 matmuls cost two array passes); PSUM accumulation
and bias adds stay fp32. Inputs arrive in 4 packed blobs + one 4 MB a2
stream on a second DMA queue. The e4 contraction streams a2 as the moving
operand against 16-col stationaries [zh_g | zl_g] (fp16 hi/lo split of ZG,
lo pre-scaled by 2^10 to stay fp16-normal), col-tiled 4-wide across the PE
array. Measured error vs the fp32 reference: ~4e-4 of output scale.
"""

import numpy as np
from contextlib import ExitStack

import concourse.bass as bass
import concourse.tile as tile
from concourse import bacc, mybir
from concourse.bass_utils import run_bass_kernel_spmd

N_CORES = 8
N, E, G, H = 4096, 8192, 64, 128
NODE_DIM, EDGE_DIM = 11, 5
G_PER_CORE = G // N_CORES          # 8 graph slots per core
CAP = 192                          # edge slots per graph (64-aligned segments)
EP = G_PER_CORE * CAP              # 1536 edge slots per core
NT = EP // 128                     # 12 edge tiles per core
NCH = EP // 512                    # 3 512-wide chunks for the feature-major MLPs
NG4 = NT // 4                      # 3 groups of 4 tiles for the edge-major stage
COLT = True                        # col-tile the final contraction 4-wide

f32 = mybir.dt.float32
f16 = mybir.dt.float16
AF = mybir.ActivationFunctionType
OP = mybir.AluOpType

# wblob column map (f16 weights packed into one [128, 1024] blob)
W_P2, W_E2, W_E30, W_E31, W_BR, W_P1, W_E1 = 0, 128, 384, 512, 640, 768, 896
# bias32 column map ([128, 16] f32)
B_P1, B_E1, B_E2, B_WME = 0, 1, 2, 4


def _slot_segments(s):
    """(tile, p0, p1) segments of graph slot s in the (p, t) edge grid."""
    segs, a, end = [], s * CAP, (s + 1) * CAP
    while a < end:
        t, p0 = divmod(a, 128)
        take = min(128 - p0, end - a)
        segs.append((t, p0, p0 + take))
        a += take
    return segs


def _emit(nc, tc, io):
    es = ExitStack()
    const = es.enter_context(tc.tile_pool(name="const", bufs=1))
    big = es.enter_context(tc.tile_pool(name="big", bufs=1))
    work = es.enter_context(tc.tile_pool(name="work", bufs=4))
    psA = es.enter_context(tc.tile_pool(name="psA", bufs=2, space="PSUM"))
    psB = es.enter_context(tc.tile_pool(name="psB", bufs=2, space="PSUM"))
    psZ = es.enter_context(tc.tile_pool(name="psZ", bufs=2, space="PSUM"))
    psO = es.enter_context(tc.tile_pool(name="psO", bufs=1, space="PSUM"))
    psR = es.enter_context(tc.tile_pool(name="psR", bufs=1, space="PSUM"))

    with es:
        # small loads first on sync; the 4 MB a2 split across the sync and
        # gpsimd queues AFTER them (a queued DMA blocks its engine's
        # instruction stream until the transfer completes)
        a2 = const.tile([128, 128 * H], f16, tag="a2")
        nc.gpsimd.dma_start(a2[:], io["a2h"][:, :])
        bias32 = const.tile([128, 16], f32, tag="bias32")
        nc.sync.dma_start(bias32[:], io["bias32"][:, :])
        rows32 = const.tile([1, 1024], f32, tag="rows32")
        nc.sync.dma_start(rows32[:], io["rows32"][:, :])
        wblob = const.tile([128, 1024], f16, tag="wblob")
        nc.sync.dma_start(wblob[:], io["wblob"][:, :])
        eaT = const.tile([EDGE_DIM, EP], f16, tag="eaT")
        nc.sync.dma_start(eaT[:], io["edge16"][0:EDGE_DIM, :])
        xsT = const.tile([NODE_DIM, EP], f16, tag="xsT")
        nc.sync.dma_start(xsT[:], io["edge16"][EDGE_DIM:EDGE_DIM + NODE_DIM, :])

        # broadcast per-output-column biases to all partitions (512-wide)
        ones_r = const.tile([1, 128], f32, tag="ones_r")
        nc.vector.memset(ones_r[:], 1.0)
        pbc = psA.tile([128, 512], f32, tag="mlp")
        nc.tensor.matmul(pbc[:], ones_r[:], rows32[:, 0:512], start=True, stop=True)
        p2bb = const.tile([128, 512], f32, tag="p2bb")
        nc.scalar.copy(p2bb[:], pbc[:])
        pbc2 = psA.tile([128, 512], f32, tag="mlp")
        nc.tensor.matmul(pbc2[:], ones_r[:], rows32[:, 512:1024], start=True,
                         stop=True)
        e3bb = const.tile([128, 512], f32, tag="e3bb")
        nc.scalar.copy(e3bb[:], pbc2[:])

        # ---- feature-major MLP interiors (epilogues split ACT/DVE) ----------
        relu1 = big.tile([128, EP], f16, tag="relu1")
        e1o = big.tile([128, EP], f16, tag="e1o")
        e2o0 = big.tile([128, EP], f16, tag="e2o0")
        e2o1 = big.tile([128, EP], f16, tag="e2o1")
        for q in range(NCH):
            sl = slice(q * 512, (q + 1) * 512)
            ps = psA.tile([128, 512], f32, tag="mlp")
            nc.tensor.matmul(ps[:], wblob[0:EDGE_DIM, W_E1:W_E1 + 128],
                             eaT[:, sl], start=True, stop=True)
            nc.scalar.activation(e1o[:, sl], ps[:], AF.Relu,
                                 bias=bias32[:, B_E1:B_E1 + 1])
            ps2 = psA.tile([128, 512], f32, tag="mlp")
            nc.tensor.matmul(ps2[:], wblob[0:NODE_DIM, W_P1:W_P1 + 128],
                             xsT[:, sl], start=True, stop=True)
            nc.vector.tensor_scalar(relu1[:, sl], ps2[:],
                                    bias32[:, B_P1:B_P1 + 1], 0.0,
                                    op0=OP.add, op1=OP.max)
        for m, e2o in enumerate((e2o0, e2o1)):
            for q in range(NCH):
                sl = slice(q * 512, (q + 1) * 512)
                ps = psA.tile([128, 512], f32, tag="mlp")
                nc.tensor.matmul(ps[:], wblob[:, W_E2 + m * 128:W_E2 + (m + 1) * 128],
                                 e1o[:, sl], start=True, stop=True)
                if q % 2 == 0:
                    nc.scalar.activation(e2o[:, sl], ps[:], AF.Relu,
                                         bias=bias32[:, B_E2 + m:B_E2 + m + 1])
                else:
                    nc.vector.tensor_scalar(e2o[:, sl], ps[:],
                                            bias32[:, B_E2 + m:B_E2 + m + 1],
                                            0.0, op0=OP.add, op1=OP.max)

        # ---- edge-major last layers, grouped 4 tiles per 512-wide epilogue --
        # h_big[e, t, i] = w_e * (relu1[:,e].T @ p2w + p2b)
        # e3x_big[e, t, k] = relu(e2o[:,e].T @ e3w + e3b); col H = 1.0
        h_big = big.tile([128, NT, H], f16, tag="hbig")
        e3x = big.tile([128, NT, H + 1], f16, tag="e3x")
        nc.gpsimd.memset(e3x[:, :, H:H + 1], 1.0)
        for g4 in range(NG4):
            psh = psB.tile([128, 512], f32, tag="he4")
            pse = psA.tile([128, 512], f32, tag="mlp")
            for j in range(4):
                t = g4 * 4 + j
                sl = slice(t * 128, (t + 1) * 128)
                jj = slice(j * 128, (j + 1) * 128)
                nc.tensor.matmul(psh[:, jj], relu1[:, sl], wblob[:, W_P2:W_P2 + 128],
                                 start=True, stop=True)
                nc.tensor.matmul(pse[:, jj], e2o0[:, sl], wblob[:, W_E30:W_E30 + 128],
                                 start=True, stop=False)
                nc.tensor.matmul(pse[:, jj], e2o1[:, sl], wblob[:, W_E31:W_E31 + 128],
                                 start=False, stop=True)
            h4 = work.tile([128, 512], f32, tag="h4")
            nc.vector.tensor_tensor(h4[:], psh[:], p2bb[:], op=OP.add)
            for j in range(4):
                t = g4 * 4 + j
                if j % 2 == 0:
                    nc.scalar.activation(h_big[:, t, :],
                                         h4[:, j * 128:(j + 1) * 128], AF.Copy,
                                         scale=bias32[:, B_WME + t:B_WME + t + 1])
                else:
                    nc.vector.tensor_scalar_mul(h_big[:, t, :],
                                                h4[:, j * 128:(j + 1) * 128],
                                                bias32[:, B_WME + t:B_WME + t + 1])
            t4 = work.tile([128, 512], f32, tag="h4")
            nc.vector.tensor_tensor(t4[:], pse[:], e3bb[:], op=OP.add)
            nc.vector.tensor_scalar_max(e3x[:, 4 * g4:4 * g4 + 4, 0:H], t4[:], 0.0)

        # ---- per-graph ZG accumulation + fp16 hi/lo split ---------------------
        # zg2[:, 0:8, k] = zh, zg2[:, 8:16, k] = 1024*zl (host scales back)
        zg2 = big.tile([128, 2 * G_PER_CORE, H], f16, tag="zg2")
        hw_f = work.tile([128, G_PER_CORE], f16, tag="hwf")
        for s in range(G_PER_CORE):
            segs = _slot_segments(s)
            pz = psZ.tile([128, H + 1], f32, tag="zg")
            for n, (t, p0, p1) in enumerate(segs):
                nc.tensor.matmul(pz[:], h_big[p0:p1, t, :], e3x[p0:p1, t, :],
                                 start=(n == 0), stop=(n == len(segs) - 1))
            nc.scalar.copy(zg2[:, s, :], pz[:, 0:H])
            zhf = work.tile([128, H], f32, tag="zhf")
            nc.scalar.activation(zhf[:], zg2[:, s, :], AF.Copy, scale=1024.0)
            nc.vector.scalar_tensor_tensor(zg2[:, G_PER_CORE + s, :],
                                           pz[:, 0:H], 1024.0, zhf[:],
                                           op0=OP.mult, op1=OP.subtract)
            nc.vector.tensor_copy(hw_f[:, s:s + 1], pz[:, H:H + 1])

        # ---- final a2 contraction: a2 streams as the moving operand ----------
        ot = work.tile([128, 128], f32, tag="ot")
        nc.gpsimd.memset(ot[:], 0.0)
        if COLT:
            po = psO.tile([128, 128], f32, tag="out")
            for k4 in range(H // 4):
                for j in range(4):
                    k = k4 * 4 + j
                    nc.tensor.matmul(po[32 * j:32 * j + 16, :], zg2[:, :, k],
                                     a2[:, k * 128:(k + 1) * 128],
                                     start=(k4 == 0), stop=(k4 == H // 4 - 1),
                                     tile_position=(0, 32 * j))
            for j in range(4):
                nc.scalar.copy(ot[32 * j:32 * j + 16, :], po[32 * j:32 * j + 16, :])
        else:
            po = psO.tile([2 * G_PER_CORE, 128], f32, tag="out")
            for k in range(H):
                nc.tensor.matmul(po[:], zg2[:, :, k],
                                 a2[:, k * 128:(k + 1) * 128],
                                 start=(k == 0), stop=(k == H - 1))
            nc.scalar.copy(ot[0:2 * G_PER_CORE, :], po[:])
        pr = psR.tile([G_PER_CORE, 128], f32, tag="br")
        nc.tensor.matmul(pr[:], hw_f[:], wblob[:, W_BR:W_BR + 128],
                         start=True, stop=True)
        ot2 = work.tile([G_PER_CORE, 128], f32, tag="ot2")
        nc.scalar.copy(ot2[:], pr[:])
        nc.sync.dma_start(io["pooled"][0:128, :], ot[:])
        nc.sync.dma_start(io["pooled"][128:128 + G_PER_CORE, :], ot2[:])


_CACHE = {}


def _build():
    if "nc" in _CACHE:
        return _CACHE["nc"]
    nc = bacc.Bacc("TRN2", target_bir_lowering=False, debug=False,
                   num_devices=N_CORES)
    io = {}

    def din(name, shape, dt=f32):
        io[name] = nc.dram_tensor(name, shape, dt, kind="ExternalInput").ap()

    din("edge16", [16, EP], f16)
    din("wblob", [128, 1024], f16)
    din("bias32", [128, 16])
    din("rows32", [1, 1024])
    din("a2h", [128, 128 * H], f16)
    io["pooled"] = nc.dram_tensor("pooled", [128 + G_PER_CORE, H], f32,
                                  kind="ExternalOutput").ap()

    with tile.TileContext(nc) as tc:
        _emit(nc, tc, io)
    nc.compile()
    _CACHE["nc"] = nc
    return nc


def _host_prep(inputs):
    x = np.asarray(inputs["x"], dtype=np.float32)
    ea = np.asarray(inputs["edge_attr"], dtype=np.float32)
    ei = np.asarray(inputs["edge_index"]).astype(np.int64)
    batch = np.asarray(inputs["batch"]).astype(np.int64)
    src, dst = ei[0], ei[1]
    gid = batch[dst]
    cnt = np.bincount(dst, minlength=N).astype(np.float32)
    w_all = 1.0 / np.maximum(cnt, 1.0)

    a2h = np.ascontiguousarray(
        np.asarray(inputs["e4_w"], np.float32)
        .reshape(128, 128, 128).transpose(1, 0, 2).reshape(128, 128 * H)
        .astype(np.float16))

    wblob = np.zeros((128, 1024), np.float16)
    wblob[:, W_P2:W_P2 + 128] = np.asarray(inputs["p2_w"], np.float16)
    wblob[:, W_E2:W_E2 + 256] = np.asarray(inputs["e2_w"], np.float16)
    wblob[:, W_E30:W_E30 + 128] = np.asarray(inputs["e3_w"], np.float16)[0:128]
    wblob[:, W_E31:W_E31 + 128] = np.asarray(inputs["e3_w"], np.float16)[128:256]
    wblob[:, W_BR:W_BR + 128] = (
        np.asarray(inputs["e4_b"], np.float32).reshape(128, 128).astype(np.float16))
    wblob[0:NODE_DIM, W_P1:W_P1 + 128] = np.asarray(inputs["p1_w"], np.float16)
    wblob[0:EDGE_DIM, W_E1:W_E1 + 128] = np.asarray(inputs["e1_w"], np.float16)

    rows32 = np.zeros((1, 1024), np.float32)
    rows32[0, 0:512] = np.tile(np.asarray(inputs["p2_b"], np.float32), 4)
    rows32[0, 512:1024] = np.tile(np.asarray(inputs["e3_b"], np.float32), 4)

    bias_c = np.zeros((128, 16), np.float32)
    bias_c[:, B_P1] = np.asarray(inputs["p1_b"], np.float32)
    bias_c[:, B_E1] = np.asarray(inputs["e1_b"], np.float32)
    bias_c[:, B_E2:B_E2 + 2] = np.asarray(
        inputs["e2_b"], np.float32).reshape(2, 128).T

    com = {"wblob": wblob, "rows32": rows32, "a2h": a2h}
    com = {k: np.ascontiguousarray(v) for k, v in com.items()}

    in_maps = []
    for c in range(N_CORES):
        ea_s = np.zeros((EP, EDGE_DIM), np.float32)
        xs_s = np.zeros((EP, NODE_DIM), np.float32)
        w_s = np.zeros(EP, np.float32)
        for s in range(G_PER_CORE):
            es = np.where(gid == c * G_PER_CORE + s)[0]
            assert len(es) <= CAP, f"graph {c * G_PER_CORE + s}: {len(es)} edges"
            pos = s * CAP + np.arange(len(es))
            ea_s[pos] = ea[es]
            xs_s[pos] = x[src[es]]
            w_s[pos] = w_all[dst[es]]

        edge16 = np.zeros((16, EP), np.float16)
        edge16[0:EDGE_DIM] = ea_s.T
        edge16[EDGE_DIM:EDGE_DIM + NODE_DIM] = xs_s.T
        b = bias_c.copy()
        b[:, B_WME:B_WME + NT] = w_s.reshape(NT, 128).T

        m = dict(com)
        m["edge16"] = np.ascontiguousarray(edge16)
        m["bias32"] = np.ascontiguousarray(b)
        in_maps.append(m)
    return in_maps


def _run(inputs, trace=False, tmpdir=None):
    nc = _build()
    in_maps = _host_prep(inputs)
    if trace:
        # No egress in this sandbox: neutralize the artifact upload the
        # trace path performs after NTFF capture, and register the NTFF
        # hook module if the image's antenv package lacks axon_hooks.
        from concourse import bass_utils as _bu
        _bu.upload_artifacts = lambda d: d
        try:
            from antenv import axon_hooks  # noqa: F401
        except ImportError:
            import importlib.util, sys as _sys
            spec = importlib.util.spec_from_file_location(
                "antenv.axon_hooks", "/opt/trn_rl_repo/antenv/axon_hooks.py")
            mod = importlib.util.module_from_spec(spec)
            spec.loader.exec_module(mod)
            _sys.modules["antenv.axon_hooks"] = mod
    res = run_bass_kernel_spmd(nc, in_maps, list(range(N_CORES)),
                               trace=trace, tmpdir=tmpdir)
    out = np.empty((G, H), np.float32)
    for c in range(N_CORES):
        p = res.results[c]["pooled"]
        acc = p[128:128 + G_PER_CORE].astype(np.float32).copy()
        ngrp = 4 if COLT else 1
        for j in range(ngrp):
            acc += p[32 * j:32 * j + G_PER_CORE]
            acc += p[32 * j + G_PER_CORE:32 * j + 2 * G_PER_CORE] * (1.0 / 1024.0)
        out[c * G_PER_CORE:(c + 1) * G_PER_CORE, :] = acc
    return out, res


def kernel(**inputs) -> np.ndarray:
    out, _ = _run(inputs)
    return out


# revision 15
# speedup vs baseline: 1.0136x; 1.0136x over previous
"""Trainium2 Bass kernel for nn_Net_16174846837292 (NNConv GNN message passing).

Strategy (graph-sharded, aggregation-folded, single fp16 a2 pass):
  pooled[g,o] = sum_{e: batch[dst[e]]=g} w_e * msg[e,o],  w_e = 1/max(cnt[dst_e],1)
  msg[e,o]    = sum_{k,i} e3[e,k]*h[src_e,i]*e4w[k,i*128+o] + sum_i h[src_e,i]*e4b[i*128+o]
  => pooled[g,o] = sum_k ZG_g[:,k]^T A2f[:,k*128+o] + HW_g^T Br
     ZG_g[i,k] = sum_{e in g} (w_e h[src_e,i]) e3[e,k],  HW_g[i] = sum_e w_e h[src_e,i]

Sharding: edges grouped by the graph of their destination node; 8 graphs per
core, so in-degree weights are per-edge host constants and NO collectives are
needed. Per-core edges pack into 8 slots of 192 (64-aligned segments).

Host precomputes w_e and pre-gathers x[src_e] per edge slot, so the device
kernel needs no histogram, no h DRAM round-trip, no indirect DMA, and no PE
transposes: the last layer of each MLP is computed edge-major by using the
previous layer's activations as the matmul stationary operand. All PE
operands are 16-bit (fp32 matmuls cost two array passes); PSUM accumulation
and bias adds stay fp32. Inputs arrive in 4 packed blobs + one 4 MB a2
stream on a second DMA queue. The e4 contraction streams a2 as the moving
operand against 16-col stationaries [zh_g | zl_g] (fp16 hi/lo split of ZG,
lo pre-scaled by 2^10 to stay fp16-normal), col-tiled 4-wide across the PE
array. Measured error vs the fp32 reference: ~4e-4 of output scale.
"""

import numpy as np
from contextlib import ExitStack

import concourse.bass as bass
import concourse.tile as tile
from concourse import bacc, mybir
from concourse.bass_utils import run_bass_kernel_spmd

N_CORES = 8
N, E, G, H = 4096, 8192, 64, 128
NODE_DIM, EDGE_DIM = 11, 5
G_PER_CORE = G // N_CORES          # 8 graph slots per core
CAP = 192                          # edge slots per graph (64-aligned segments)
EP = G_PER_CORE * CAP              # 1536 edge slots per core
NT = EP // 128                     # 12 edge tiles per core
NCH = EP // 512                    # 3 512-wide chunks for the feature-major MLPs
NG4 = NT // 4                      # 3 groups of 4 tiles for the edge-major stage
COLT = True                        # col-tile the final contraction 4-wide

f32 = mybir.dt.float32
f16 = mybir.dt.float16
AF = mybir.ActivationFunctionType
OP = mybir.AluOpType

# wblob column map (f16 weights packed into one [128, 1024] blob)
W_P2, W_E2, W_E30, W_E31, W_BR, W_P1, W_E1 = 0, 128, 384, 512, 640, 768, 896
# bias32 column map ([128, 16] f32)
B_P1, B_E1, B_E2, B_WME = 0, 1, 2, 4


def _slot_segments(s):
    """(tile, p0, p1) segments of graph slot s in the (p, t) edge grid."""
    segs, a, end = [], s * CAP, (s + 1) * CAP
    while a < end:
        t, p0 = divmod(a, 128)
        take = min(128 - p0, end - a)
        segs.append((t, p0, p0 + take))
        a += take
    return segs


def _emit(nc, tc, io):
    es = ExitStack()
    const = es.enter_context(tc.tile_pool(name="const", bufs=1))
    a2pool = es.enter_context(tc.tile_pool(name="a2pool", bufs=1))
    big = es.enter_context(tc.tile_pool(name="big", bufs=1))
    work = es.enter_context(tc.tile_pool(name="work", bufs=4))
    psA = es.enter_context(tc.tile_pool(name="psA", bufs=2, space="PSUM"))
    psB = es.enter_context(tc.tile_pool(name="psB", bufs=2, space="PSUM"))
    psZ = es.enter_context(tc.tile_pool(name="psZ", bufs=2, space="PSUM"))
    psO = es.enter_context(tc.tile_pool(name="psO", bufs=1, space="PSUM"))
    psR = es.enter_context(tc.tile_pool(name="psR", bufs=1, space="PSUM"))

    with es:
        # small loads first on sync; the 4 MB a2 split across the sync and
        # gpsimd queues AFTER them (a queued DMA blocks its engine's
        # instruction stream until the transfer completes)
        a2 = a2pool.tile([128, 128 * H], f16, tag="a2")
        nc.gpsimd.dma_start(a2[:], io["a2h"][:, :])
        bias32 = const.tile([128, 16], f32, tag="bias32")
        nc.sync.dma_start(bias32[:], io["bias32"][:, :])
        rows32 = const.tile([1, 1024], f32, tag="rows32")
        nc.sync.dma_start(rows32[:], io["rows32"][:, :])
        wblob = const.tile([128, 1024], f16, tag="wblob")
        nc.sync.dma_start(wblob[:], io["wblob"][:, :])
        eaT = const.tile([EDGE_DIM, EP], f16, tag="eaT")
        nc.sync.dma_start(eaT[:], io["edge16"][0:EDGE_DIM, :])
        xsT = const.tile([NODE_DIM, EP], f16, tag="xsT")
        nc.sync.dma_start(xsT[:], io["edge16"][EDGE_DIM:EDGE_DIM + NODE_DIM, :])

        # broadcast per-output-column biases to all partitions (512-wide)
        ones_r = const.tile([1, 128], f32, tag="ones_r")
        nc.vector.memset(ones_r[:], 1.0)
        pbc = psA.tile([128, 512], f32, tag="mlp")
        nc.tensor.matmul(pbc[:], ones_r[:], rows32[:, 0:512], start=True, stop=True)
        p2bb = const.tile([128, 512], f32, tag="p2bb")
        nc.scalar.copy(p2bb[:], pbc[:])
        pbc2 = psA.tile([128, 512], f32, tag="mlp")
        nc.tensor.matmul(pbc2[:], ones_r[:], rows32[:, 512:1024], start=True,
                         stop=True)
        e3bb = const.tile([128, 512], f32, tag="e3bb")
        nc.scalar.copy(e3bb[:], pbc2[:])

        # ---- feature-major MLP interiors (epilogues split ACT/DVE) ----------
        relu1 = big.tile([128, EP], f16, tag="relu1")
        e1o = big.tile([128, EP], f16, tag="e1o")
        e2o0 = big.tile([128, EP], f16, tag="e2o0")
        e2o1 = big.tile([128, EP], f16, tag="e2o1")
        for q in range(NCH):
            sl = slice(q * 512, (q + 1) * 512)
            ps = psA.tile([128, 512], f32, tag="mlp")
            nc.tensor.matmul(ps[:], wblob[0:EDGE_DIM, W_E1:W_E1 + 128],
                             eaT[:, sl], start=True, stop=True)
            nc.scalar.activation(e1o[:, sl], ps[:], AF.Relu,
                                 bias=bias32[:, B_E1:B_E1 + 1])
            ps2 = psA.tile([128, 512], f32, tag="mlp")
            nc.tensor.matmul(ps2[:], wblob[0:NODE_DIM, W_P1:W_P1 + 128],
                             xsT[:, sl], start=True, stop=True)
            nc.vector.tensor_scalar(relu1[:, sl], ps2[:],
                                    bias32[:, B_P1:B_P1 + 1], 0.0,
                                    op0=OP.add, op1=OP.max)
        for m, e2o in enumerate((e2o0, e2o1)):
            for q in range(NCH):
                sl = slice(q * 512, (q + 1) * 512)
                ps = psA.tile([128, 512], f32, tag="mlp")
                nc.tensor.matmul(ps[:], wblob[:, W_E2 + m * 128:W_E2 + (m + 1) * 128],
                                 e1o[:, sl], start=True, stop=True)
                if q % 2 == 0:
                    nc.scalar.activation(e2o[:, sl], ps[:], AF.Relu,
                                         bias=bias32[:, B_E2 + m:B_E2 + m + 1])
                else:
                    nc.vector.tensor_scalar(e2o[:, sl], ps[:],
                                            bias32[:, B_E2 + m:B_E2 + m + 1],
                                            0.0, op0=OP.add, op1=OP.max)

        # ---- edge-major last layers, grouped 4 tiles per 512-wide epilogue --
        # h_big[e, t, i] = w_e * (relu1[:,e].T @ p2w + p2b)
        # e3x_big[e, t, k] = relu(e2o[:,e].T @ e3w + e3b); col H = 1.0
        h_big = big.tile([128, NT, H], f16, tag="hbig")
        e3x = big.tile([128, NT, H + 1], f16, tag="e3x")
        nc.gpsimd.memset(e3x[:, :, H:H + 1], 1.0)
        for g4 in range(NG4):
            psh = psB.tile([128, 512], f32, tag="he4")
            pse = psA.tile([128, 512], f32, tag="mlp")
            for j in range(4):
                t = g4 * 4 + j
                sl = slice(t * 128, (t + 1) * 128)
                jj = slice(j * 128, (j + 1) * 128)
                nc.tensor.matmul(psh[:, jj], relu1[:, sl], wblob[:, W_P2:W_P2 + 128],
                                 start=True, stop=True)
                nc.tensor.matmul(pse[:, jj], e2o0[:, sl], wblob[:, W_E30:W_E30 + 128],
                                 start=True, stop=False)
                nc.tensor.matmul(pse[:, jj], e2o1[:, sl], wblob[:, W_E31:W_E31 + 128],
                                 start=False, stop=True)
            h4 = work.tile([128, 512], f32, tag="h4")
            nc.vector.tensor_tensor(h4[:], psh[:], p2bb[:], op=OP.add)
            for j in range(4):
                t = g4 * 4 + j
                if j % 2 == 0:
                    nc.scalar.activation(h_big[:, t, :],
                                         h4[:, j * 128:(j + 1) * 128], AF.Copy,
                                         scale=bias32[:, B_WME + t:B_WME + t + 1])
                else:
                    nc.vector.tensor_scalar_mul(h_big[:, t, :],
                                                h4[:, j * 128:(j + 1) * 128],
                                                bias32[:, B_WME + t:B_WME + t + 1])
            t4 = work.tile([128, 512], f32, tag="h4")
            nc.vector.tensor_tensor(t4[:], pse[:], e3bb[:], op=OP.add)
            nc.vector.tensor_scalar_max(e3x[:, 4 * g4:4 * g4 + 4, 0:H], t4[:], 0.0)

        # ---- per-graph ZG accumulation + fp16 hi/lo split ---------------------
        # zg2[:, 0:8, k] = zh, zg2[:, 8:16, k] = 1024*zl (host scales back)
        zg2 = big.tile([128, 2 * G_PER_CORE, H], f16, tag="zg2")
        hw_f = work.tile([128, G_PER_CORE], f16, tag="hwf")
        for s in range(G_PER_CORE):
            segs = _slot_segments(s)
            pz = psZ.tile([128, H + 1], f32, tag="zg")
            for n, (t, p0, p1) in enumerate(segs):
                nc.tensor.matmul(pz[:], h_big[p0:p1, t, :], e3x[p0:p1, t, :],
                                 start=(n == 0), stop=(n == len(segs) - 1))
            nc.scalar.copy(zg2[:, s, :], pz[:, 0:H])
            zhf = work.tile([128, H], f32, tag="zhf")
            nc.scalar.activation(zhf[:], zg2[:, s, :], AF.Copy, scale=1024.0)
            nc.vector.scalar_tensor_tensor(zg2[:, G_PER_CORE + s, :],
                                           pz[:, 0:H], 1024.0, zhf[:],
                                           op0=OP.mult, op1=OP.subtract)
            nc.vector.tensor_copy(hw_f[:, s:s + 1], pz[:, H:H + 1])

        # ---- final a2 contraction: a2 streams as the moving operand ----------
        ot = work.tile([128, 128], f32, tag="ot")
        nc.gpsimd.memset(ot[:], 0.0)
        if COLT:
            po = psO.tile([128, 128], f32, tag="out")
            for k4 in range(H // 4):
                for j in range(4):
                    k = k4 * 4 + j
                    nc.tensor.matmul(po[32 * j:32 * j + 16, :], zg2[:, :, k],
                                     a2[:, k * 128:(k + 1) * 128],
                                     start=(k4 == 0), stop=(k4 == H // 4 - 1),
                                     tile_position=(0, 32 * j))
            for j in range(4):
                nc.scalar.copy(ot[32 * j:32 * j + 16, :], po[32 * j:32 * j + 16, :])
        else:
            po = psO.tile([2 * G_PER_CORE, 128], f32, tag="out")
            for k in range(H):
                nc.tensor.matmul(po[:], zg2[:, :, k],
                                 a2[:, k * 128:(k + 1) * 128],
                                 start=(k == 0), stop=(k == H - 1))
            nc.scalar.copy(ot[0:2 * G_PER_CORE, :], po[:])
        pr = psR.tile([G_PER_CORE, 128], f32, tag="br")
        nc.tensor.matmul(pr[:], hw_f[:], wblob[:, W_BR:W_BR + 128],
                         start=True, stop=True)
        ot2 = work.tile([G_PER_CORE, 128], f32, tag="ot2")
        nc.scalar.copy(ot2[:], pr[:])
        nc.sync.dma_start(io["pooled"][0:128, :], ot[:])
        nc.sync.dma_start(io["pooled"][128:128 + G_PER_CORE, :], ot2[:])


_CACHE = {}


def _build():
    if "nc" in _CACHE:
        return _CACHE["nc"]
    nc = bacc.Bacc("TRN2", target_bir_lowering=False, debug=False,
                   num_devices=N_CORES)
    io = {}

    def din(name, shape, dt=f32):
        io[name] = nc.dram_tensor(name, shape, dt, kind="ExternalInput").ap()

    din("edge16", [16, EP], f16)
    din("wblob", [128, 1024], f16)
    din("bias32", [128, 16])
    din("rows32", [1, 1024])
    din("a2h", [128, 128 * H], f16)
    io["pooled"] = nc.dram_tensor("pooled", [128 + G_PER_CORE, H], f32,
                                  kind="ExternalOutput").ap()

    with tile.TileContext(nc) as tc:
        _emit(nc, tc, io)
    nc.compile()
    _CACHE["nc"] = nc
    return nc


def _host_prep(inputs):
    x = np.asarray(inputs["x"], dtype=np.float32)
    ea = np.asarray(inputs["edge_attr"], dtype=np.float32)
    ei = np.asarray(inputs["edge_index"]).astype(np.int64)
    batch = np.asarray(inputs["batch"]).astype(np.int64)
    src, dst = ei[0], ei[1]
    gid = batch[dst]
    cnt = np.bincount(dst, minlength=N).astype(np.float32)
    w_all = 1.0 / np.maximum(cnt, 1.0)

    a2h = np.ascontiguousarray(
        np.asarray(inputs["e4_w"], np.float32)
        .reshape(128, 128, 128).transpose(1, 0, 2).reshape(128, 128 * H)
        .astype(np.float16))

    wblob = np.zeros((128, 1024), np.float16)
    wblob[:, W_P2:W_P2 + 128] = np.asarray(inputs["p2_w"], np.float16)
    wblob[:, W_E2:W_E2 + 256] = np.asarray(inputs["e2_w"], np.float16)
    wblob[:, W_E30:W_E30 + 128] = np.asarray(inputs["e3_w"], np.float16)[0:128]
    wblob[:, W_E31:W_E31 + 128] = np.asarray(inputs["e3_w"], np.float16)[128:256]
    wblob[:, W_BR:W_BR + 128] = (
        np.asarray(inputs["e4_b"], np.float32).reshape(128, 128).astype(np.float16))
    wblob[0:NODE_DIM, W_P1:W_P1 + 128] = np.asarray(inputs["p1_w"], np.float16)
    wblob[0:EDGE_DIM, W_E1:W_E1 + 128] = np.asarray(inputs["e1_w"], np.float16)

    rows32 = np.zeros((1, 1024), np.float32)
    rows32[0, 0:512] = np.tile(np.asarray(inputs["p2_b"], np.float32), 4)
    rows32[0, 512:1024] = np.tile(np.asarray(inputs["e3_b"], np.float32), 4)

    bias_c = np.zeros((128, 16), np.float32)
    bias_c[:, B_P1] = np.asarray(inputs["p1_b"], np.float32)
    bias_c[:, B_E1] = np.asarray(inputs["e1_b"], np.float32)
    bias_c[:, B_E2:B_E2 + 2] = np.asarray(
        inputs["e2_b"], np.float32).reshape(2, 128).T

    com = {"wblob": wblob, "rows32": rows32, "a2h": a2h}
    com = {k: np.ascontiguousarray(v) for k, v in com.items()}

    in_maps = []
    for c in range(N_CORES):
        ea_s = np.zeros((EP, EDGE_DIM), np.float32)
        xs_s = np.zeros((EP, NODE_DIM), np.float32)
        w_s = np.zeros(EP, np.float32)
        for s in range(G_PER_CORE):
            es = np.where(gid == c * G_PER_CORE + s)[0]
            assert len(es) <= CAP, f"graph {c * G_PER_CORE + s}: {len(es)} edges"
            pos = s * CAP + np.arange(len(es))
            ea_s[pos] = ea[es]
            xs_s[pos] = x[src[es]]
            w_s[pos] = w_all[dst[es]]

        edge16 = np.zeros((16, EP), np.float16)
        edge16[0:EDGE_DIM] = ea_s.T
        edge16[EDGE_DIM:EDGE_DIM + NODE_DIM] = xs_s.T
        b = bias_c.copy()
        b[:, B_WME:B_WME + NT] = w_s.reshape(NT, 128).T

        m = dict(com)
        m["edge16"] = np.ascontiguousarray(edge16)
        m["bias32"] = np.ascontiguousarray(b)
        in_maps.append(m)
    return in_maps


def _run(inputs, trace=False, tmpdir=None):
    nc = _build()
    in_maps = _host_prep(inputs)
    if trace:
        # No egress in this sandbox: neutralize the artifact upload the
        # trace path performs after NTFF capture, and register the NTFF
        # hook module if the image's antenv package lacks axon_hooks.
        from concourse import bass_utils as _bu
        _bu.upload_artifacts = lambda d: d
        try:
            from antenv import axon_hooks  # noqa: F401
        except ImportError:
            import importlib.util, sys as _sys
            spec = importlib.util.spec_from_file_location(
                "antenv.axon_hooks", "/opt/trn_rl_repo/antenv/axon_hooks.py")
            mod = importlib.util.module_from_spec(spec)
            spec.loader.exec_module(mod)
            _sys.modules["antenv.axon_hooks"] = mod
    res = run_bass_kernel_spmd(nc, in_maps, list(range(N_CORES)),
                               trace=trace, tmpdir=tmpdir)
    out = np.empty((G, H), np.float32)
    for c in range(N_CORES):
        p = res.results[c]["pooled"]
        acc = p[128:128 + G_PER_CORE].astype(np.float32).copy()
        ngrp = 4 if COLT else 1
        for j in range(ngrp):
            acc += p[32 * j:32 * j + G_PER_CORE]
            acc += p[32 * j + G_PER_CORE:32 * j + 2 * G_PER_CORE] * (1.0 / 1024.0)
        out[c * G_PER_CORE:(c + 1) * G_PER_CORE, :] = acc
    return out, res


def kernel(**inputs) -> np.ndarray:
    out, _ = _run(inputs)
    return out


# revision 24
# speedup vs baseline: 1.1963x; 1.1803x over previous
"""Trainium2 Bass kernel for nn_Net_16174846837292 (NNConv GNN message passing).

Strategy (graph-sharded, aggregation-folded, single fp16 a2 pass):
  pooled[g,o] = sum_{e: batch[dst[e]]=g} w_e * msg[e,o],  w_e = 1/max(cnt[dst_e],1)
  msg[e,o]    = sum_{k,i} e3[e,k]*h[src_e,i]*e4w[k,i*128+o] + sum_i h[src_e,i]*e4b[i*128+o]
  => pooled[g,o] = sum_k ZG_g[:,k]^T A2f[:,k*128+o] + HW_g^T Br
     ZG_g[i,k] = sum_{e in g} (w_e h[src_e,i]) e3[e,k],  HW_g[i] = sum_e w_e h[src_e,i]

Sharding: edges grouped by the graph of their destination node; 8 graphs per
core, so in-degree weights are per-edge host constants and NO collectives are
needed. Per-core edges pack into 8 slots of 192 (64-aligned segments).

Key folds that keep the device pipeline lean:
- host pre-gathers x[src_e], scales it by w_e and appends a w_e row, against
  [p1w; p1b]: node-MLP epilogues are then plain ReLU / plain copies.
- p2_b's contribution separates into p2b[i]*S_g[k] (S_g[k]=sum_e w_e e3[e,k])
  plus W_g*(p2b@Br): the first is one extra matmul S^T @ B2 (B2 host-built)
  accumulated into the final PSUM, the second is added on the host.
- e3_b is injected with ones-row matmuls into the e3 PSUM accumulation.
- all PE operands are 16-bit (fp32 matmuls cost two array passes); PSUM and
  epilogue arithmetic stay fp32.
- ZG splits into bf16 hi/lo halves [zh|zl] (bf16 = fp32 exponent range, no
  subnormal trouble); the 4 MB fp16 a2 streams once as the moving operand
  against those 16-col stationaries, col-tiled 4-wide across the PE array.
- big a2 DMAs are gated behind the small input loads (they'd monopolize the
  16 SDMA engines and stall everything), and tiny warmup matmuls keep the
  PE HAM clock at 2.4 GHz while inputs land.

Measured error vs the fp32 reference: ~4e-4 of output scale.
"""

import numpy as np
from contextlib import ExitStack

import concourse.bass as bass
import concourse.tile as tile
from concourse import bacc, mybir
from concourse.bass_utils import run_bass_kernel_spmd

N_CORES = 8
N, E, G, H = 4096, 8192, 64, 128
NODE_DIM, EDGE_DIM = 11, 5
G_PER_CORE = G // N_CORES          # 8 graph slots per core
CAP = 192                          # edge slots per graph (64-aligned segments)
EP = G_PER_CORE * CAP              # 1536 edge slots per core
NT = EP // 128                     # 12 edge tiles per core
NCH = EP // 512                    # 3 512-wide chunks for the feature-major MLPs
XD = NODE_DIM + 1                  # xs rows: w*x (11) + w (1)

f32 = mybir.dt.float32
f16 = mybir.dt.float16
bf16 = mybir.dt.bfloat16
AF = mybir.ActivationFunctionType
OP = mybir.AluOpType

# wblob column map (f16 weights packed into one [128, 1152] blob)
W_P2, W_E2, W_E30, W_E31, W_BR, W_P1, W_E1, W_B2 = (
    0, 128, 384, 512, 640, 768, 896, 1024)
WBW = 1152
# bias32 column map ([128, 4] f32)
B_E1, B_E2 = 0, 1


def _slot_segments(s):
    """(tile, p0, p1) segments of graph slot s in the (p, t) edge grid."""
    segs, a, end = [], s * CAP, (s + 1) * CAP
    while a < end:
        t, p0 = divmod(a, 128)
        take = min(128 - p0, end - a)
        segs.append((t, p0, p0 + take))
        a += take
    return segs


def _emit(nc, tc, io):
    es = ExitStack()
    const = es.enter_context(tc.tile_pool(name="const", bufs=1))
    a2pool = es.enter_context(tc.tile_pool(name="a2pool", bufs=1))
    big = es.enter_context(tc.tile_pool(name="big", bufs=1))
    work = es.enter_context(tc.tile_pool(name="work", bufs=4))
    psA = es.enter_context(tc.tile_pool(name="psA", bufs=2, space="PSUM"))
    psB = es.enter_context(tc.tile_pool(name="psB", bufs=2, space="PSUM"))
    psZ = es.enter_context(tc.tile_pool(name="psZ", bufs=3, space="PSUM"))
    psO = es.enter_context(tc.tile_pool(name="psO", bufs=1, space="PSUM"))

    with es:
        a2 = a2pool.tile([128, 128 * H], f16, tag="a2")

        wblob = const.tile([128, WBW], f16, tag="wblob")
        nc.sync.dma_start(wblob[:], io["wblob"][:, :])
        eaT = const.tile([EDGE_DIM, EP], f16, tag="eaT")
        nc.sync.dma_start(eaT[:], io["edge16"][0:EDGE_DIM, :])
        xsT = const.tile([XD, EP], f16, tag="xsT")
        nc.sync.dma_start(xsT[:], io["edge16"][EDGE_DIM:EDGE_DIM + XD, :])
        wme = const.tile([128, NT], f16, tag="wme")
        nc.sync.dma_start(wme[:], io["wme"][:, :])
        bias32 = const.tile([128, 4], f32, tag="bias32")
        nc.sync.dma_start(bias32[:], io["bias32"][:, :])
        rows16 = const.tile([1, 128], f16, tag="rows16")
        last_dma = nc.sync.dma_start(rows16[:], io["rows16"][:, :])

        # a2 halves on the idle sync/gpsimd queues, gated behind the last
        # small input load: a big transfer issued early monopolizes the 16
        # SDMA engines and delays every small input DMA behind it
        KSYNC = 72 * 128
        a2d0 = nc.sync.dma_start(a2[:, 0:KSYNC], io["a2h"][:, 0:KSYNC])
        a2d1 = nc.gpsimd.dma_start(a2[:, KSYNC:128 * H], io["a2h"][:, KSYNC:128 * H])
        for ad in (a2d0, a2d1):
            bass._add_dep_helper(ad.ins, last_dma.ins, sync=True,
                                 reason="a2 after small input DMAs")

        # spin tiny matmuls while input DMAs land so HAM is warm (2.4 GHz)
        # when the real pipeline starts
        ones_r = const.tile([1, 128], f16, tag="ones_r")
        nc.vector.memset(ones_r[:], 1.0)
        for _ in range(56):
            pw = psA.tile([128, 512], f32, tag="mlp")
            nc.tensor.matmul(pw[0:16, 0:16], ones_r[:, 0:16], ones_r[:, 0:16],
                             start=True, stop=True)

        # ---- feature-major MLP interiors (epilogues split ACT/DVE) ----------
        relu1 = big.tile([128, EP], f16, tag="relu1")
        e1o = big.tile([128, EP], f16, tag="e1o")
        e2o0 = big.tile([128, EP], f16, tag="e2o0")
        e2o1 = big.tile([128, EP], f16, tag="e2o1")
        for q in range(NCH):
            sl = slice(q * 512, (q + 1) * 512)
            ps = psA.tile([128, 512], f32, tag="mlp")
            nc.tensor.matmul(ps[:], wblob[0:EDGE_DIM, W_E1:W_E1 + 128],
                             eaT[:, sl], start=True, stop=True)
            nc.scalar.activation(e1o[:, sl], ps[:], AF.Relu,
                                 bias=bias32[:, B_E1:B_E1 + 1])
            ps2 = psA.tile([128, 512], f32, tag="mlp")
            nc.tensor.matmul(ps2[:], wblob[0:XD, W_P1:W_P1 + 128],
                             xsT[:, sl], start=True, stop=True)
            nc.vector.tensor_scalar_max(relu1[:, sl], ps2[:], 0.0)
        for q in range(NCH):
            for m, e2o in enumerate((e2o0, e2o1)):
                sl = slice(q * 512, (q + 1) * 512)
                ps = psA.tile([128, 512], f32, tag="mlp")
                nc.tensor.matmul(ps[:], wblob[:, W_E2 + m * 128:W_E2 + (m + 1) * 128],
                                 e1o[:, sl], start=True, stop=True)
                if m == 0:
                    nc.scalar.activation(e2o[:, sl], ps[:], AF.Relu,
                                         bias=bias32[:, B_E2 + m:B_E2 + m + 1])
                else:
                    nc.vector.tensor_scalar(e2o[:, sl], ps[:],
                                            bias32[:, B_E2 + m:B_E2 + m + 1],
                                            0.0, op0=OP.add, op1=OP.max)

        # ---- edge-major last layers, 4-tile groups, plain epilogues ---------
        # h_big[e, t, i] = relu1[:, e].T @ p2w      (w_e, p1b already folded)
        # e3x[e, t, k]   = relu(e2o[:, e].T @ e3w + e3b); col H = 1.0
        h_big = big.tile([128, NT, H], f16, tag="hbig")
        e3x = big.tile([128, NT, H + 1], f16, tag="e3x")
        nc.gpsimd.memset(e3x[:, :, H:H + 1], 1.0)
        for g4 in range(NT // 4):
            psh = psB.tile([128, 512], f32, tag="he4")
            pse = psA.tile([128, 512], f32, tag="mlp")
            for j in range(4):
                t = g4 * 4 + j
                sl = slice(t * 128, (t + 1) * 128)
                jj = slice(j * 128, (j + 1) * 128)
                nc.tensor.matmul(psh[:, jj], relu1[:, sl], wblob[:, W_P2:W_P2 + 128],
                                 start=True, stop=True)
                nc.tensor.matmul(pse[:, jj], e2o0[:, sl], wblob[:, W_E30:W_E30 + 128],
                                 start=True, stop=False)
                nc.tensor.matmul(pse[:, jj], e2o1[:, sl], wblob[:, W_E31:W_E31 + 128],
                                 start=False, stop=False)
                nc.tensor.matmul(pse[:, jj], ones_r[:], rows16[:],
                                 start=False, stop=True)
            if g4 % 2 == 0:
                nc.scalar.copy(h_big[:, 4 * g4:4 * g4 + 4, :], psh[:])
                nc.vector.tensor_scalar_max(e3x[:, 4 * g4:4 * g4 + 4, 0:H],
                                            pse[:], 0.0)
            else:
                nc.vector.tensor_copy(h_big[:, 4 * g4:4 * g4 + 4, :], psh[:])
                nc.scalar.activation(e3x[:, 4 * g4:4 * g4 + 4, 0:H], pse[:],
                                     AF.Relu)

        # ---- per-graph ZG accumulation + bf16 hi/lo split --------------------
        # pz col 0:128 = ZG (stationary h), col 128 = HW (ones col of e3x),
        # col 129 = S_g[k] (stationary e3x, moving w_e col)
        zg2 = big.tile([128, 2 * G_PER_CORE, H], bf16, tag="zg2")
        hw_f = work.tile([128, 2 * G_PER_CORE], f16, tag="hwf")
        nc.vector.memset(hw_f[:, G_PER_CORE:2 * G_PER_CORE], 0.0)
        s_f = work.tile([128, G_PER_CORE], f16, tag="sf")
        for s in range(G_PER_CORE):
            segs = _slot_segments(s)
            pz = psZ.tile([128, H + 2], f32, tag="zg")
            # the S chain must start only after the ZG chain closes: a
            # matmul with start=True clears has_written for its partitions
            # across the whole PSUM region, killing any open chain there
            for n, (t, p0, p1) in enumerate(segs):
                nc.tensor.matmul(pz[:, 0:H + 1], h_big[p0:p1, t, :],
                                 e3x[p0:p1, t, :],
                                 start=(n == 0), stop=(n == len(segs) - 1))
            for n, (t, p0, p1) in enumerate(segs):
                nc.tensor.matmul(pz[:, H + 1:H + 2], e3x[p0:p1, t, 0:H],
                                 wme[p0:p1, t:t + 1],
                                 start=(n == 0), stop=(n == len(segs) - 1))
            if s % 2 == 0:
                nc.scalar.copy(zg2[:, s, :], pz[:, 0:H])
            else:
                nc.vector.tensor_copy(zg2[:, s, :], pz[:, 0:H])
            nc.vector.tensor_tensor(zg2[:, G_PER_CORE + s, :], pz[:, 0:H],
                                    zg2[:, s, :], op=OP.subtract)
            nc.scalar.copy(hw_f[:, s:s + 1], pz[:, H:H + 1])
            nc.scalar.copy(s_f[:, s:s + 1], pz[:, H + 1:H + 2])

        # ---- final a2 contraction: a2 streams as the moving operand ----------
        # bias terms (e4b via hw_f@Br, p2b via S^T@B2) ride as the last
        # accumulation steps of col-group 0 (hw_f cols 8:16 are zeros)
        ot = work.tile([128, 128], f32, tag="ot")
        nc.gpsimd.memset(ot[:], 0.0)
        po = psO.tile([128, 128], f32, tag="out")
        for k4 in range(H // 4):
            for j in range(4):
                k = k4 * 4 + j
                last = (k4 == H // 4 - 1) and (j != 0)
                nc.tensor.matmul(po[32 * j:32 * j + 16, :], zg2[:, :, k],
                                 a2[:, k * 128:(k + 1) * 128],
                                 start=(k4 == 0), stop=last,
                                 tile_position=(0, 32 * j))
        nc.tensor.matmul(po[0:16, :], hw_f[:], wblob[:, W_BR:W_BR + 128],
                         start=False, stop=False, tile_position=(0, 0))
        nc.tensor.matmul(po[0:G_PER_CORE, :], s_f[:], wblob[:, W_B2:W_B2 + 128],
                         start=False, stop=True, tile_position=(0, 0))
        for j in range(4):
            nc.scalar.copy(ot[32 * j:32 * j + 16, :], po[32 * j:32 * j + 16, :])
        nc.sync.dma_start(io["pooled"][:, :], ot[:])


_CACHE = {}


def _build():
    if "nc" in _CACHE:
        return _CACHE["nc"]
    nc = bacc.Bacc("TRN2", target_bir_lowering=False, debug=False,
                   num_devices=N_CORES)
    io = {}

    def din(name, shape, dt=f32):
        io[name] = nc.dram_tensor(name, shape, dt, kind="ExternalInput").ap()

    din("edge16", [EDGE_DIM + XD, EP], f16)
    din("wblob", [128, WBW], f16)
    din("wme", [128, NT], f16)
    din("bias32", [128, 4])
    din("rows16", [1, 128], f16)
    din("a2h", [128, 128 * H], f16)
    io["pooled"] = nc.dram_tensor("pooled", [128, H], f32,
                                  kind="ExternalOutput").ap()

    with tile.TileContext(nc) as tc:
        _emit(nc, tc, io)
    nc.compile()
    _CACHE["nc"] = nc
    return nc


def _host_prep(inputs):
    x = np.asarray(inputs["x"], dtype=np.float32)
    ea = np.asarray(inputs["edge_attr"], dtype=np.float32)
    ei = np.asarray(inputs["edge_index"]).astype(np.int64)
    batch = np.asarray(inputs["batch"]).astype(np.int64)
    src, dst = ei[0], ei[1]
    gid = batch[dst]
    cnt = np.bincount(dst, minlength=N).astype(np.float32)
    w_all = 1.0 / np.maximum(cnt, 1.0)

    e4w = np.asarray(inputs["e4_w"], np.float32).reshape(128, 128, 128)
    a2h = np.ascontiguousarray(
        e4w.transpose(1, 0, 2).reshape(128, 128 * H).astype(np.float16))
    p2b = np.asarray(inputs["p2_b"], np.float32)
    b2 = np.einsum("i,kio->ko", p2b, e4w).astype(np.float16)   # [k, o]
    br = np.asarray(inputs["e4_b"], np.float32).reshape(128, 128)
    br2 = p2b @ br                                             # [o]

    wblob = np.zeros((128, WBW), np.float16)
    wblob[:, W_P2:W_P2 + 128] = np.asarray(inputs["p2_w"], np.float16)
    wblob[:, W_E2:W_E2 + 256] = np.asarray(inputs["e2_w"], np.float16)
    wblob[:, W_E30:W_E30 + 128] = np.asarray(inputs["e3_w"], np.float16)[0:128]
    wblob[:, W_E31:W_E31 + 128] = np.asarray(inputs["e3_w"], np.float16)[128:256]
    wblob[:, W_BR:W_BR + 128] = br.astype(np.float16)
    wblob[0:NODE_DIM, W_P1:W_P1 + 128] = np.asarray(inputs["p1_w"], np.float16)
    wblob[NODE_DIM, W_P1:W_P1 + 128] = np.asarray(inputs["p1_b"], np.float16)
    wblob[0:EDGE_DIM, W_E1:W_E1 + 128] = np.asarray(inputs["e1_w"], np.float16)
    wblob[:, W_B2:W_B2 + 128] = b2

    rows16 = np.asarray(inputs["e3_b"], np.float16).reshape(1, 128)

    bias_c = np.zeros((128, 4), np.float32)
    bias_c[:, B_E1] = np.asarray(inputs["e1_b"], np.float32)
    bias_c[:, B_E2:B_E2 + 2] = np.asarray(
        inputs["e2_b"], np.float32).reshape(2, 128).T

    com = {"wblob": wblob, "rows16": rows16, "bias32": bias_c, "a2h": a2h}
    com = {k: np.ascontiguousarray(v) for k, v in com.items()}

    in_maps = []
    wg_all = np.zeros((N_CORES, G_PER_CORE), np.float32)
    for c in range(N_CORES):
        ea_s = np.zeros((EP, EDGE_DIM), np.float32)
        xs_s = np.zeros((EP, XD), np.float32)
        w_s = np.zeros(EP, np.float32)
        for s in range(G_PER_CORE):
            es = np.where(gid == c * G_PER_CORE + s)[0]
            assert len(es) <= CAP, f"graph {c * G_PER_CORE + s}: {len(es)} edges"
            pos = s * CAP + np.arange(len(es))
            we = w_all[dst[es]]
            ea_s[pos] = ea[es]
            xs_s[pos, 0:NODE_DIM] = x[src[es]] * we[:, None]
            xs_s[pos, NODE_DIM] = we
            w_s[pos] = we
            wg_all[c, s] = we.sum()

        edge16 = np.zeros((EDGE_DIM + XD, EP), np.float16)
        edge16[0:EDGE_DIM] = ea_s.T
        edge16[EDGE_DIM:EDGE_DIM + XD] = xs_s.T

        m = dict(com)
        m["edge16"] = np.ascontiguousarray(edge16)
        m["wme"] = np.ascontiguousarray(w_s.reshape(NT, 128).T.astype(np.float16))
        in_maps.append(m)
    return in_maps, wg_all, br2


def _run(inputs, trace=False, tmpdir=None):
    nc = _build()
    in_maps, wg_all, br2 = _host_prep(inputs)
    if trace:
        # No egress in this sandbox: neutralize the artifact upload the
        # trace path performs after NTFF capture, and register the NTFF
        # hook module if the image's antenv package lacks axon_hooks.
        from concourse import bass_utils as _bu
        _bu.upload_artifacts = lambda d: d
        try:
            from antenv import axon_hooks  # noqa: F401
        except ImportError:
            import importlib.util, sys as _sys
            spec = importlib.util.spec_from_file_location(
                "antenv.axon_hooks", "/opt/trn_rl_repo/antenv/axon_hooks.py")
            mod = importlib.util.module_from_spec(spec)
            spec.loader.exec_module(mod)
            _sys.modules["antenv.axon_hooks"] = mod
    res = run_bass_kernel_spmd(nc, in_maps, list(range(N_CORES)),
                               trace=trace, tmpdir=tmpdir)
    out = np.empty((G, H), np.float32)
    for c in range(N_CORES):
        p = res.results[c]["pooled"]
        acc = np.zeros((G_PER_CORE, H), np.float32)
        for j in range(4):
            acc += p[32 * j:32 * j + G_PER_CORE]
            acc += p[32 * j + G_PER_CORE:32 * j + 2 * G_PER_CORE]
        acc += wg_all[c][:, None] * br2[None, :]   # W_g * (p2b @ Br)
        out[c * G_PER_CORE:(c + 1) * G_PER_CORE, :] = acc
    return out, res


def kernel(**inputs) -> np.ndarray:
    out, _ = _run(inputs)
    return out


# revision 27
# speedup vs baseline: 1.2667x; 1.0588x over previous
"""Trainium2 Bass kernel for nn_Net_16174846837292 (NNConv GNN message passing).

Strategy (graph-sharded, aggregation-folded, single fp16 a2 pass):
  pooled[g,o] = sum_{e: batch[dst[e]]=g} w_e * msg[e,o],  w_e = 1/max(cnt[dst_e],1)
  msg[e,o]    = sum_{k,i} e3[e,k]*h[src_e,i]*e4w[k,i*128+o] + sum_i h[src_e,i]*e4b[i*128+o]
  => pooled[g,o] = sum_k ZG_g[:,k]^T A2f[:,k*128+o] + HW_g^T Br
     ZG_g[i,k] = sum_{e in g} (w_e h[src_e,i]) e3[e,k],  HW_g[i] = sum_e w_e h[src_e,i]

Sharding: edges grouped by the graph of their destination node; 8 graphs per
core, so in-degree weights are per-edge host constants and NO collectives are
needed. Per-core edges pack into 8 slots of 192 (64-aligned segments).

Device pipeline (all PE operands 16-bit; fp32 would cost two array passes):
- host pre-gathers x[src_e], scales it by w_e and appends a w_e row, matched
  against [p1w; p1b]: the node MLP needs no bias handling and no per-tile
  w_e scaling on the device at all.
- last layer of each MLP runs edge-major (previous activations as the matmul
  stationary operand): no transposes, no gathers anywhere.
- ZG accumulates per graph slot in PSUM ([ZG | HW] via a ones column), then
  splits into bf16 hi/lo halves [zh|zl] (bf16 = fp32 exponent range, no
  subnormal trouble).
- the 4 MB fp16 a2 streams once as the moving operand against the 16-col
  [zh|zl] stationaries, col-tiled 4-wide across the PE array; the e4-bias
  term rides the same accumulation as hw_f @ Br.
- big a2 DMAs are gated behind the small input loads (a big transfer issued
  early monopolizes the 16 SDMA engines and stalls every input behind it),
  and tiny warmup matmuls keep the PE HAM clock at 2.4 GHz while they land.

Measured error vs the fp32 reference: ~4e-4 of output scale.
"""

import numpy as np
from contextlib import ExitStack

import concourse.bass as bass
import concourse.tile as tile
from concourse import bacc, mybir
from concourse.bass_utils import run_bass_kernel_spmd

N_CORES = 8
N, E, G, H = 4096, 8192, 64, 128
NODE_DIM, EDGE_DIM = 11, 5
G_PER_CORE = G // N_CORES          # 8 graph slots per core
CAP = 192                          # edge slots per graph (64-aligned segments)
EP = G_PER_CORE * CAP              # 1536 edge slots per core
NT = EP // 128                     # 12 edge tiles per core
NCH = EP // 512                    # 3 512-wide chunks for the feature-major MLPs
XD = NODE_DIM + 1                  # xs rows: w*x (11) + w (1)

f32 = mybir.dt.float32
f16 = mybir.dt.float16
bf16 = mybir.dt.bfloat16
AF = mybir.ActivationFunctionType
OP = mybir.AluOpType

# wblob column map (f16 weights packed into one [128, 1024] blob)
W_P2, W_E2, W_E30, W_E31, W_BR, W_P1, W_E1, W_B2 = (
    0, 128, 384, 512, 640, 768, 896, 1024)
WBW = 1152
# bias32 column map ([128, 4] f32)
B_E1, B_E2 = 0, 1


def _slot_segments(s):
    """(tile, p0, p1) segments of graph slot s in the (p, t) edge grid."""
    segs, a, end = [], s * CAP, (s + 1) * CAP
    while a < end:
        t, p0 = divmod(a, 128)
        take = min(128 - p0, end - a)
        segs.append((t, p0, p0 + take))
        a += take
    return segs


def _emit(nc, tc, io):
    es = ExitStack()
    const = es.enter_context(tc.tile_pool(name="const", bufs=1))
    a2pool = es.enter_context(tc.tile_pool(name="a2pool", bufs=1))
    big = es.enter_context(tc.tile_pool(name="big", bufs=1))
    work = es.enter_context(tc.tile_pool(name="work", bufs=4))
    psA = es.enter_context(tc.tile_pool(name="psA", bufs=2, space="PSUM"))
    psB = es.enter_context(tc.tile_pool(name="psB", bufs=2, space="PSUM"))
    psZ = es.enter_context(tc.tile_pool(name="psZ", bufs=3, space="PSUM"))
    psO = es.enter_context(tc.tile_pool(name="psO", bufs=1, space="PSUM"))

    with es:
        a2 = a2pool.tile([128, 128 * H], f16, tag="a2")

        wblob = const.tile([128, WBW], f16, tag="wblob")
        nc.sync.dma_start(wblob[:], io["wblob"][:, :])
        eaT = const.tile([EDGE_DIM, EP], f16, tag="eaT")
        nc.sync.dma_start(eaT[:], io["edge16"][0:EDGE_DIM, :])
        xsT = const.tile([XD, EP], f16, tag="xsT")
        nc.sync.dma_start(xsT[:], io["edge16"][EDGE_DIM:EDGE_DIM + XD, :])
        wme = const.tile([128, NT], f16, tag="wme")
        nc.sync.dma_start(wme[:], io["wme"][:, :])
        bias32 = const.tile([128, 4], f32, tag="bias32")
        nc.sync.dma_start(bias32[:], io["bias32"][:, :])
        rows16 = const.tile([1, 512], f16, tag="rows16")
        last_dma = nc.sync.dma_start(rows16[:], io["rows16"][:, :])

        # a2 halves on the idle sync/gpsimd queues, gated behind the last
        # small input load: a big transfer issued early monopolizes the 16
        # SDMA engines and delays every small input DMA behind it
        KSYNC = 72 * 128
        a2d0 = nc.sync.dma_start(a2[:, 0:KSYNC], io["a2h"][:, 0:KSYNC])
        a2d1 = nc.gpsimd.dma_start(a2[:, KSYNC:128 * H], io["a2h"][:, KSYNC:128 * H])
        for ad in (a2d0, a2d1):
            bass._add_dep_helper(ad.ins, last_dma.ins, sync=True,
                                 reason="a2 after small input DMAs")

        # spin tiny matmuls while input DMAs land so HAM is warm (2.4 GHz)
        # when the real pipeline starts
        ones_r = const.tile([1, 128], f16, tag="ones_r")
        nc.vector.memset(ones_r[:], 1.0)
        for _ in range(56):
            pw = psA.tile([128, 512], f32, tag="mlp")
            nc.tensor.matmul(pw[0:16, 0:16], ones_r[:, 0:16], ones_r[:, 0:16],
                             start=True, stop=True)

        # ---- feature-major MLP interiors (epilogues split ACT/DVE) ----------
        relu1 = big.tile([128, EP], f16, tag="relu1")
        e1o = big.tile([128, EP], f16, tag="e1o")
        e2o0 = big.tile([128, EP], f16, tag="e2o0")
        e2o1 = big.tile([128, EP], f16, tag="e2o1")
        for q in range(NCH):
            sl = slice(q * 512, (q + 1) * 512)
            ps = psA.tile([128, 512], f32, tag="mlp")
            nc.tensor.matmul(ps[:], wblob[0:EDGE_DIM, W_E1:W_E1 + 128],
                             eaT[:, sl], start=True, stop=True)
            nc.scalar.activation(e1o[:, sl], ps[:], AF.Relu,
                                 bias=bias32[:, B_E1:B_E1 + 1])
            ps2 = psA.tile([128, 512], f32, tag="mlp")
            nc.tensor.matmul(ps2[:], wblob[0:XD, W_P1:W_P1 + 128],
                             xsT[:, sl], start=True, stop=True)
            nc.vector.tensor_scalar_max(relu1[:, sl], ps2[:], 0.0)
        for q in range(NCH):
            for m, e2o in enumerate((e2o0, e2o1)):
                sl = slice(q * 512, (q + 1) * 512)
                ps = psA.tile([128, 512], f32, tag="mlp")
                nc.tensor.matmul(ps[:], wblob[:, W_E2 + m * 128:W_E2 + (m + 1) * 128],
                                 e1o[:, sl], start=True, stop=True)
                if m == 0:
                    nc.scalar.activation(e2o[:, sl], ps[:], AF.Relu,
                                         bias=bias32[:, B_E2 + m:B_E2 + m + 1])
                else:
                    nc.vector.tensor_scalar(e2o[:, sl], ps[:],
                                            bias32[:, B_E2 + m:B_E2 + m + 1],
                                            0.0, op0=OP.add, op1=OP.max)

        # broadcast e3b (tiled 4x) to all partitions for 512-wide adds
        pbc2 = psA.tile([128, 512], f32, tag="mlp")
        nc.tensor.matmul(pbc2[:], ones_r[:], rows16[0:1, 0:512],
                         start=True, stop=True)
        e3bb = const.tile([128, 512], f32, tag="e3bb")
        nc.scalar.copy(e3bb[:], pbc2[:])

        # ---- edge-major last layers, 4-tile groups, 512-wide epilogues ------
        # h_big[e, t, i] = relu1[:, e].T @ p2w + p2b  (w_e, p1b folded on host)
        # e3x[e, t, k]   = relu(e2o[:, e].T @ e3w + e3b); col H = 1.0
        h_big = big.tile([128, NT, H], f16, tag="hbig")
        e3x = big.tile([128, NT, H + 1], f16, tag="e3x")
        nc.gpsimd.memset(e3x[:, :, H:H + 1], 1.0)
        for g4 in range(NT // 4):
            psh = psB.tile([128, 512], f32, tag="he4")
            pse = psA.tile([128, 512], f32, tag="mlp")
            for j in range(4):
                t = g4 * 4 + j
                sl = slice(t * 128, (t + 1) * 128)
                jj = slice(j * 128, (j + 1) * 128)
                nc.tensor.matmul(psh[:, jj], relu1[:, sl], wblob[:, W_P2:W_P2 + 128],
                                 start=True, stop=True)
                nc.tensor.matmul(pse[:, jj], e2o0[:, sl], wblob[:, W_E30:W_E30 + 128],
                                 start=True, stop=False)
                nc.tensor.matmul(pse[:, jj], e2o1[:, sl], wblob[:, W_E31:W_E31 + 128],
                                 start=False, stop=True)
            g4s = slice(4 * g4, 4 * g4 + 4)
            if g4 % 2 == 0:
                nc.scalar.copy(h_big[:, g4s, :], psh[:])
            else:
                nc.vector.tensor_copy(h_big[:, g4s, :], psh[:])
            t4 = work.tile([128, 512], f32, tag="t4")
            nc.vector.tensor_tensor(t4[:], pse[:], e3bb[:], op=OP.add)
            nc.scalar.activation(e3x[:, g4s, 0:H], t4[:], AF.Relu)

        # ---- per-graph ZG accumulation + bf16 hi/lo split --------------------
        # pz cols 0:128 = ZG (stationary h), col 128 = HW (ones col of e3x)
        zg2 = big.tile([128, 2 * G_PER_CORE, H], bf16, tag="zg2")
        hw_f = work.tile([128, 2 * G_PER_CORE], f16, tag="hwf")
        nc.vector.memset(hw_f[:, G_PER_CORE:2 * G_PER_CORE], 0.0)
        s_f = work.tile([128, G_PER_CORE], f16, tag="sf")
        for s in range(G_PER_CORE):
            segs = _slot_segments(s)
            pz = psZ.tile([128, H + 2], f32, tag="zg")
            # the S chain starts only after the ZG chain closes: a matmul
            # with start=True clears has_written for its partitions across
            # the whole PSUM region, killing any open chain there
            for n, (t, p0, p1) in enumerate(segs):
                nc.tensor.matmul(pz[:, 0:H + 1], h_big[p0:p1, t, :],
                                 e3x[p0:p1, t, :],
                                 start=(n == 0), stop=(n == len(segs) - 1))
            for n, (t, p0, p1) in enumerate(segs):
                nc.tensor.matmul(pz[:, H + 1:H + 2], e3x[p0:p1, t, 0:H],
                                 wme[p0:p1, t:t + 1],
                                 start=(n == 0), stop=(n == len(segs) - 1))
            if s % 2 == 0:
                nc.scalar.copy(zg2[:, s, :], pz[:, 0:H])
            else:
                nc.vector.tensor_copy(zg2[:, s, :], pz[:, 0:H])
            nc.vector.tensor_tensor(zg2[:, G_PER_CORE + s, :], pz[:, 0:H],
                                    zg2[:, s, :], op=OP.subtract)
            nc.scalar.copy(hw_f[:, s:s + 1], pz[:, H:H + 1])
            nc.scalar.copy(s_f[:, s:s + 1], pz[:, H + 1:H + 2])

        # ---- final a2 contraction: a2 streams as the moving operand ----------
        # the e4-bias term (hw_f @ Br) rides as the last accumulation step of
        # col-group 0 (hw_f cols 8:16 are zeros)
        ot = work.tile([128, 128], f32, tag="ot")
        nc.gpsimd.memset(ot[:], 0.0)
        po = psO.tile([128, 128], f32, tag="out")
        for k4 in range(H // 4):
            for j in range(4):
                k = k4 * 4 + j
                last = (k4 == H // 4 - 1) and (j != 0)
                nc.tensor.matmul(po[32 * j:32 * j + 16, :], zg2[:, :, k],
                                 a2[:, k * 128:(k + 1) * 128],
                                 start=(k4 == 0), stop=last,
                                 tile_position=(0, 32 * j))
        nc.tensor.matmul(po[0:16, :], hw_f[:], wblob[:, W_BR:W_BR + 128],
                         start=False, stop=False, tile_position=(0, 0))
        nc.tensor.matmul(po[0:G_PER_CORE, :], s_f[:], wblob[:, W_B2:W_B2 + 128],
                         start=False, stop=True, tile_position=(0, 0))
        for j in range(4):
            nc.scalar.copy(ot[32 * j:32 * j + 16, :], po[32 * j:32 * j + 16, :])
        nc.sync.dma_start(io["pooled"][:, :], ot[:])


_CACHE = {}


def _build():
    if "nc" in _CACHE:
        return _CACHE["nc"]
    nc = bacc.Bacc("TRN2", target_bir_lowering=False, debug=False,
                   num_devices=N_CORES)
    io = {}

    def din(name, shape, dt=f32):
        io[name] = nc.dram_tensor(name, shape, dt, kind="ExternalInput").ap()

    din("edge16", [EDGE_DIM + XD, EP], f16)
    din("wblob", [128, WBW], f16)
    din("wme", [128, NT], f16)
    din("bias32", [128, 4])
    din("rows16", [1, 512], f16)
    din("a2h", [128, 128 * H], f16)
    io["pooled"] = nc.dram_tensor("pooled", [128, H], f32,
                                  kind="ExternalOutput").ap()

    with tile.TileContext(nc) as tc:
        _emit(nc, tc, io)
    nc.compile()
    _CACHE["nc"] = nc
    return nc


def _host_prep(inputs):
    x = np.asarray(inputs["x"], dtype=np.float32)
    ea = np.asarray(inputs["edge_attr"], dtype=np.float32)
    ei = np.asarray(inputs["edge_index"]).astype(np.int64)
    batch = np.asarray(inputs["batch"]).astype(np.int64)
    src, dst = ei[0], ei[1]
    gid = batch[dst]
    cnt = np.bincount(dst, minlength=N).astype(np.float32)
    w_all = 1.0 / np.maximum(cnt, 1.0)

    e4w = np.asarray(inputs["e4_w"], np.float32).reshape(128, 128, 128)
    a2h = np.ascontiguousarray(
        e4w.transpose(1, 0, 2).reshape(128, 128 * H).astype(np.float16))
    p2b = np.asarray(inputs["p2_b"], np.float32)
    b2 = np.einsum("i,kio->ko", p2b, e4w).astype(np.float16)   # [k, o]
    br = np.asarray(inputs["e4_b"], np.float32).reshape(128, 128)
    br2 = p2b @ br                                             # [o]

    wblob = np.zeros((128, WBW), np.float16)
    wblob[:, W_P2:W_P2 + 128] = np.asarray(inputs["p2_w"], np.float16)
    wblob[:, W_E2:W_E2 + 256] = np.asarray(inputs["e2_w"], np.float16)
    wblob[:, W_E30:W_E30 + 128] = np.asarray(inputs["e3_w"], np.float16)[0:128]
    wblob[:, W_E31:W_E31 + 128] = np.asarray(inputs["e3_w"], np.float16)[128:256]
    wblob[:, W_BR:W_BR + 128] = br.astype(np.float16)
    wblob[:, W_B2:W_B2 + 128] = b2
    wblob[0:NODE_DIM, W_P1:W_P1 + 128] = np.asarray(inputs["p1_w"], np.float16)
    wblob[NODE_DIM, W_P1:W_P1 + 128] = np.asarray(inputs["p1_b"], np.float16)
    wblob[0:EDGE_DIM, W_E1:W_E1 + 128] = np.asarray(inputs["e1_w"], np.float16)

    rows16 = np.zeros((1, 512), np.float16)
    rows16[0, :] = np.tile(np.asarray(inputs["e3_b"], np.float16), 4)

    bias_c = np.zeros((128, 4), np.float32)
    bias_c[:, B_E1] = np.asarray(inputs["e1_b"], np.float32)
    bias_c[:, B_E2:B_E2 + 2] = np.asarray(
        inputs["e2_b"], np.float32).reshape(2, 128).T

    com = {"wblob": wblob, "rows16": rows16, "bias32": bias_c, "a2h": a2h}
    com = {k: np.ascontiguousarray(v) for k, v in com.items()}

    in_maps = []
    wg_all = np.zeros((N_CORES, G_PER_CORE), np.float32)
    for c in range(N_CORES):
        ea_s = np.zeros((EP, EDGE_DIM), np.float32)
        xs_s = np.zeros((EP, XD), np.float32)
        w_s = np.zeros(EP, np.float32)
        for s in range(G_PER_CORE):
            es = np.where(gid == c * G_PER_CORE + s)[0]
            assert len(es) <= CAP, f"graph {c * G_PER_CORE + s}: {len(es)} edges"
            pos = s * CAP + np.arange(len(es))
            we = w_all[dst[es]]
            ea_s[pos] = ea[es]
            xs_s[pos, 0:NODE_DIM] = x[src[es]] * we[:, None]
            xs_s[pos, NODE_DIM] = we
            w_s[pos] = we
            wg_all[c, s] = we.sum()

        edge16 = np.zeros((EDGE_DIM + XD, EP), np.float16)
        edge16[0:EDGE_DIM] = ea_s.T
        edge16[EDGE_DIM:EDGE_DIM + XD] = xs_s.T

        m = dict(com)
        m["edge16"] = np.ascontiguousarray(edge16)
        m["wme"] = np.ascontiguousarray(w_s.reshape(NT, 128).T.astype(np.float16))
        in_maps.append(m)
    return in_maps, wg_all, br2


def _run(inputs, trace=False, tmpdir=None):
    nc = _build()
    in_maps, wg_all, br2 = _host_prep(inputs)
    if trace:
        # No egress in this sandbox: neutralize the artifact upload the
        # trace path performs after NTFF capture, and register the NTFF
        # hook module if the image's antenv package lacks axon_hooks.
        from concourse import bass_utils as _bu
        _bu.upload_artifacts = lambda d: d
        try:
            from antenv import axon_hooks  # noqa: F401
        except ImportError:
            import importlib.util, sys as _sys
            spec = importlib.util.spec_from_file_location(
                "antenv.axon_hooks", "/opt/trn_rl_repo/antenv/axon_hooks.py")
            mod = importlib.util.module_from_spec(spec)
            spec.loader.exec_module(mod)
            _sys.modules["antenv.axon_hooks"] = mod
    res = run_bass_kernel_spmd(nc, in_maps, list(range(N_CORES)),
                               trace=trace, tmpdir=tmpdir)
    out = np.empty((G, H), np.float32)
    for c in range(N_CORES):
        p = res.results[c]["pooled"]
        acc = np.zeros((G_PER_CORE, H), np.float32)
        for j in range(4):
            acc += p[32 * j:32 * j + G_PER_CORE]
            acc += p[32 * j + G_PER_CORE:32 * j + 2 * G_PER_CORE]
        acc += wg_all[c][:, None] * br2[None, :]   # W_g * (p2b @ Br)
        out[c * G_PER_CORE:(c + 1) * G_PER_CORE, :] = acc
    return out, res


def kernel(**inputs) -> np.ndarray:
    out, _ = _run(inputs)
    return out


# revision 28
# speedup vs baseline: 1.2989x; 1.0255x over previous
"""Trainium2 Bass kernel for nn_Net_16174846837292 (NNConv GNN message passing).

Strategy (graph-sharded, aggregation-folded, single fp16 a2 pass):
  pooled[g,o] = sum_{e: batch[dst[e]]=g} w_e * msg[e,o],  w_e = 1/max(cnt[dst_e],1)
  msg[e,o]    = sum_{k,i} e3[e,k]*h[src_e,i]*e4w[k,i*128+o] + sum_i h[src_e,i]*e4b[i*128+o]
  => pooled[g,o] = sum_k ZG_g[:,k]^T A2f[:,k*128+o] + HW_g^T Br
     ZG_g[i,k] = sum_{e in g} (w_e h[src_e,i]) e3[e,k],  HW_g[i] = sum_e w_e h[src_e,i]

Sharding: edges grouped by the graph of their destination node; 8 graphs per
core, so in-degree weights are per-edge host constants and NO collectives are
needed. Per-core edges pack into 8 slots of 192 (64-aligned segments).

Device pipeline (all PE operands 16-bit; fp32 would cost two array passes):
- host pre-gathers x[src_e], scales it by w_e and appends a w_e row, matched
  against [p1w; p1b]: the node MLP needs no bias handling and no per-tile
  w_e scaling on the device at all.
- last layer of each MLP runs edge-major (previous activations as the matmul
  stationary operand): no transposes, no gathers anywhere.
- ZG accumulates per graph slot in PSUM ([ZG | HW] via a ones column), then
  splits into bf16 hi/lo halves [zh|zl] (bf16 = fp32 exponent range, no
  subnormal trouble).
- the 4 MB fp16 a2 streams once as the moving operand against the 16-col
  [zh|zl] stationaries, col-tiled 4-wide across the PE array; the e4-bias
  term rides the same accumulation as hw_f @ Br.
- big a2 DMAs are gated behind the small input loads (a big transfer issued
  early monopolizes the 16 SDMA engines and stalls every input behind it),
  and tiny warmup matmuls keep the PE HAM clock at 2.4 GHz while they land.

Measured error vs the fp32 reference: ~4e-4 of output scale.
"""

import numpy as np
from contextlib import ExitStack

import concourse.bass as bass
import concourse.tile as tile
from concourse import bacc, mybir
from concourse.bass_utils import run_bass_kernel_spmd

N_CORES = 8
N, E, G, H = 4096, 8192, 64, 128
NODE_DIM, EDGE_DIM = 11, 5
G_PER_CORE = G // N_CORES          # 8 graph slots per core
CAP = 192                          # edge slots per graph (64-aligned segments)
EP = G_PER_CORE * CAP              # 1536 edge slots per core
NT = EP // 128                     # 12 edge tiles per core
NCH = EP // 512                    # 3 512-wide chunks for the feature-major MLPs
XD = NODE_DIM + 1                  # xs rows: w*x (11) + w (1)

f32 = mybir.dt.float32
f16 = mybir.dt.float16
bf16 = mybir.dt.bfloat16
AF = mybir.ActivationFunctionType
OP = mybir.AluOpType

# wblob column map (f16 weights packed into one [128, 1024] blob)
W_P2, W_E2, W_E30, W_E31, W_BR, W_P1, W_E1, W_B2 = (
    0, 128, 384, 512, 640, 768, 896, 1024)
WBW = 1152
# bias32 column map ([128, 4] f32)
B_E1, B_E2 = 0, 1


def _slot_segments(s):
    """(tile, p0, p1) segments of graph slot s in the (p, t) edge grid."""
    segs, a, end = [], s * CAP, (s + 1) * CAP
    while a < end:
        t, p0 = divmod(a, 128)
        take = min(128 - p0, end - a)
        segs.append((t, p0, p0 + take))
        a += take
    return segs


def _emit(nc, tc, io):
    es = ExitStack()
    const = es.enter_context(tc.tile_pool(name="const", bufs=1))
    a2pool = es.enter_context(tc.tile_pool(name="a2pool", bufs=1))
    big = es.enter_context(tc.tile_pool(name="big", bufs=1))
    work = es.enter_context(tc.tile_pool(name="work", bufs=4))
    psA = es.enter_context(tc.tile_pool(name="psA", bufs=2, space="PSUM"))
    psB = es.enter_context(tc.tile_pool(name="psB", bufs=2, space="PSUM"))
    psZ = es.enter_context(tc.tile_pool(name="psZ", bufs=3, space="PSUM"))
    psO = es.enter_context(tc.tile_pool(name="psO", bufs=1, space="PSUM"))

    with es:
        a2 = a2pool.tile([128, 128 * H], f16, tag="a2")

        wblob = const.tile([128, WBW], f16, tag="wblob")
        nc.sync.dma_start(wblob[:], io["wblob"][:, :])
        eaT = const.tile([EDGE_DIM, EP], f16, tag="eaT")
        nc.sync.dma_start(eaT[:], io["edge16"][0:EDGE_DIM, :])
        xsT = const.tile([XD, EP], f16, tag="xsT")
        nc.sync.dma_start(xsT[:], io["edge16"][EDGE_DIM:EDGE_DIM + XD, :])
        wme = const.tile([128, NT], f16, tag="wme")
        nc.sync.dma_start(wme[:], io["wme"][:, :])
        bias32 = const.tile([128, 4], f32, tag="bias32")
        nc.sync.dma_start(bias32[:], io["bias32"][:, :])
        rows16 = const.tile([1, 512], f16, tag="rows16")
        last_dma = nc.sync.dma_start(rows16[:], io["rows16"][:, :])

        # a2 halves on the idle sync/gpsimd queues, gated behind the last
        # small input load: a big transfer issued early monopolizes the 16
        # SDMA engines and delays every small input DMA behind it
        KSYNC = 72 * 128
        a2d0 = nc.sync.dma_start(a2[:, 0:KSYNC], io["a2h"][:, 0:KSYNC])
        a2d1 = nc.gpsimd.dma_start(a2[:, KSYNC:128 * H], io["a2h"][:, KSYNC:128 * H])
        for ad in (a2d0, a2d1):
            bass._add_dep_helper(ad.ins, last_dma.ins, sync=True,
                                 reason="a2 after small input DMAs")

        # spin tiny matmuls while input DMAs land so HAM is warm (2.4 GHz)
        # when the real pipeline starts
        ones_r = const.tile([1, 128], f16, tag="ones_r")
        nc.vector.memset(ones_r[:], 1.0)
        for _ in range(40):
            pw = psA.tile([128, 512], f32, tag="mlp")
            nc.tensor.matmul(pw[0:16, 0:16], ones_r[:, 0:16], ones_r[:, 0:16],
                             start=True, stop=True)

        # ---- feature-major MLP interiors (epilogues split ACT/DVE) ----------
        relu1 = big.tile([128, EP], f16, tag="relu1")
        e1o = big.tile([128, EP], f16, tag="e1o")
        e2o0 = big.tile([128, EP], f16, tag="e2o0")
        e2o1 = big.tile([128, EP], f16, tag="e2o1")
        for q in range(NCH):
            sl = slice(q * 512, (q + 1) * 512)
            ps = psA.tile([128, 512], f32, tag="mlp")
            nc.tensor.matmul(ps[:], wblob[0:EDGE_DIM, W_E1:W_E1 + 128],
                             eaT[:, sl], start=True, stop=True)
            nc.scalar.activation(e1o[:, sl], ps[:], AF.Relu,
                                 bias=bias32[:, B_E1:B_E1 + 1])
            ps2 = psA.tile([128, 512], f32, tag="mlp")
            nc.tensor.matmul(ps2[:], wblob[0:XD, W_P1:W_P1 + 128],
                             xsT[:, sl], start=True, stop=True)
            nc.vector.tensor_scalar_max(relu1[:, sl], ps2[:], 0.0)
        for q in range(NCH):
            for m, e2o in enumerate((e2o0, e2o1)):
                sl = slice(q * 512, (q + 1) * 512)
                ps = psA.tile([128, 512], f32, tag="mlp")
                nc.tensor.matmul(ps[:], wblob[:, W_E2 + m * 128:W_E2 + (m + 1) * 128],
                                 e1o[:, sl], start=True, stop=True)
                if m == 0:
                    nc.scalar.activation(e2o[:, sl], ps[:], AF.Relu,
                                         bias=bias32[:, B_E2 + m:B_E2 + m + 1])
                else:
                    nc.vector.tensor_scalar(e2o[:, sl], ps[:],
                                            bias32[:, B_E2 + m:B_E2 + m + 1],
                                            0.0, op0=OP.add, op1=OP.max)

        # broadcast e3b (tiled 4x) to all partitions for 512-wide adds
        pbc2 = psA.tile([128, 512], f32, tag="mlp")
        nc.tensor.matmul(pbc2[:], ones_r[:], rows16[0:1, 0:512],
                         start=True, stop=True)
        e3bb = const.tile([128, 512], f32, tag="e3bb")
        nc.scalar.copy(e3bb[:], pbc2[:])

        # ---- edge-major last layers, 4-tile groups, 512-wide epilogues ------
        # h_big[e, t, i] = relu1[:, e].T @ p2w + p2b  (w_e, p1b folded on host)
        # e3x[e, t, k]   = relu(e2o[:, e].T @ e3w + e3b); col H = 1.0
        h_big = big.tile([128, NT, H], f16, tag="hbig")
        e3x = big.tile([128, NT, H + 1], f16, tag="e3x")
        nc.gpsimd.memset(e3x[:, :, H:H + 1], 1.0)
        for g4 in range(NT // 4):
            psh = psB.tile([128, 512], f32, tag="he4")
            pse = psA.tile([128, 512], f32, tag="mlp")
            for j in range(4):
                t = g4 * 4 + j
                sl = slice(t * 128, (t + 1) * 128)
                jj = slice(j * 128, (j + 1) * 128)
                nc.tensor.matmul(psh[:, jj], relu1[:, sl], wblob[:, W_P2:W_P2 + 128],
                                 start=True, stop=True)
                nc.tensor.matmul(pse[:, jj], e2o0[:, sl], wblob[:, W_E30:W_E30 + 128],
                                 start=True, stop=False)
                nc.tensor.matmul(pse[:, jj], e2o1[:, sl], wblob[:, W_E31:W_E31 + 128],
                                 start=False, stop=True)
            g4s = slice(4 * g4, 4 * g4 + 4)
            if g4 % 2 == 0:
                nc.scalar.copy(h_big[:, g4s, :], psh[:])
            else:
                nc.vector.tensor_copy(h_big[:, g4s, :], psh[:])
            t4 = work.tile([128, 512], f32, tag="t4")
            nc.vector.tensor_tensor(t4[:], pse[:], e3bb[:], op=OP.add)
            nc.scalar.activation(e3x[:, g4s, 0:H], t4[:], AF.Relu)

        # ---- per-graph ZG accumulation + bf16 hi/lo split --------------------
        # pz cols 0:128 = ZG (stationary h), col 128 = HW (ones col of e3x)
        zg2 = big.tile([128, 2 * G_PER_CORE, H], bf16, tag="zg2")
        hw_f = work.tile([128, 2 * G_PER_CORE], f16, tag="hwf")
        nc.vector.memset(hw_f[:, G_PER_CORE:2 * G_PER_CORE], 0.0)
        s_f = work.tile([128, G_PER_CORE], f16, tag="sf")
        for s in range(G_PER_CORE):
            segs = _slot_segments(s)
            pz = psZ.tile([128, H + 2], f32, tag="zg")
            # the S chain starts only after the ZG chain closes: a matmul
            # with start=True clears has_written for its partitions across
            # the whole PSUM region, killing any open chain there
            for n, (t, p0, p1) in enumerate(segs):
                nc.tensor.matmul(pz[:, 0:H + 1], h_big[p0:p1, t, :],
                                 e3x[p0:p1, t, :],
                                 start=(n == 0), stop=(n == len(segs) - 1))
            for n, (t, p0, p1) in enumerate(segs):
                nc.tensor.matmul(pz[:, H + 1:H + 2], e3x[p0:p1, t, 0:H],
                                 wme[p0:p1, t:t + 1],
                                 start=(n == 0), stop=(n == len(segs) - 1))
            if s % 2 == 0:
                nc.scalar.copy(zg2[:, s, :], pz[:, 0:H])
            else:
                nc.vector.tensor_copy(zg2[:, s, :], pz[:, 0:H])
            nc.vector.tensor_tensor(zg2[:, G_PER_CORE + s, :], pz[:, 0:H],
                                    zg2[:, s, :], op=OP.subtract)
            nc.vector.tensor_copy(hw_f[:, s:s + 1], pz[:, H:H + 1])
            nc.vector.tensor_copy(s_f[:, s:s + 1], pz[:, H + 1:H + 2])

        # ---- final a2 contraction: a2 streams as the moving operand ----------
        # the e4-bias term (hw_f @ Br) rides as the last accumulation step of
        # col-group 0 (hw_f cols 8:16 are zeros)
        ot = work.tile([128, 128], f32, tag="ot")
        nc.gpsimd.memset(ot[:], 0.0)
        po = psO.tile([128, 128], f32, tag="out")
        for k4 in range(H // 4):
            for j in range(4):
                k = k4 * 4 + j
                last = (k4 == H // 4 - 1) and (j != 0)
                nc.tensor.matmul(po[32 * j:32 * j + 16, :], zg2[:, :, k],
                                 a2[:, k * 128:(k + 1) * 128],
                                 start=(k4 == 0), stop=last,
                                 tile_position=(0, 32 * j))
        nc.tensor.matmul(po[0:16, :], hw_f[:], wblob[:, W_BR:W_BR + 128],
                         start=False, stop=False, tile_position=(0, 0))
        nc.tensor.matmul(po[0:G_PER_CORE, :], s_f[:], wblob[:, W_B2:W_B2 + 128],
                         start=False, stop=True, tile_position=(0, 0))
        for j in range(4):
            if j % 2 == 0:
                nc.scalar.copy(ot[32 * j:32 * j + 16, :], po[32 * j:32 * j + 16, :])
            else:
                nc.vector.tensor_copy(ot[32 * j:32 * j + 16, :],
                                      po[32 * j:32 * j + 16, :])
        nc.sync.dma_start(io["pooled"][0:64, :], ot[0:64, :])
        nc.sync.dma_start(io["pooled"][64:128, :], ot[64:128, :])


_CACHE = {}


def _build():
    if "nc" in _CACHE:
        return _CACHE["nc"]
    nc = bacc.Bacc("TRN2", target_bir_lowering=False, debug=False,
                   num_devices=N_CORES)
    io = {}

    def din(name, shape, dt=f32):
        io[name] = nc.dram_tensor(name, shape, dt, kind="ExternalInput").ap()

    din("edge16", [EDGE_DIM + XD, EP], f16)
    din("wblob", [128, WBW], f16)
    din("wme", [128, NT], f16)
    din("bias32", [128, 4])
    din("rows16", [1, 512], f16)
    din("a2h", [128, 128 * H], f16)
    io["pooled"] = nc.dram_tensor("pooled", [128, H], f32,
                                  kind="ExternalOutput").ap()

    with tile.TileContext(nc) as tc:
        _emit(nc, tc, io)
    nc.compile()
    _CACHE["nc"] = nc
    return nc


def _host_prep(inputs):
    x = np.asarray(inputs["x"], dtype=np.float32)
    ea = np.asarray(inputs["edge_attr"], dtype=np.float32)
    ei = np.asarray(inputs["edge_index"]).astype(np.int64)
    batch = np.asarray(inputs["batch"]).astype(np.int64)
    src, dst = ei[0], ei[1]
    gid = batch[dst]
    cnt = np.bincount(dst, minlength=N).astype(np.float32)
    w_all = 1.0 / np.maximum(cnt, 1.0)

    e4w = np.asarray(inputs["e4_w"], np.float32).reshape(128, 128, 128)
    a2h = np.ascontiguousarray(
        e4w.transpose(1, 0, 2).reshape(128, 128 * H).astype(np.float16))
    p2b = np.asarray(inputs["p2_b"], np.float32)
    b2 = np.einsum("i,kio->ko", p2b, e4w).astype(np.float16)   # [k, o]
    br = np.asarray(inputs["e4_b"], np.float32).reshape(128, 128)
    br2 = p2b @ br                                             # [o]

    wblob = np.zeros((128, WBW), np.float16)
    wblob[:, W_P2:W_P2 + 128] = np.asarray(inputs["p2_w"], np.float16)
    wblob[:, W_E2:W_E2 + 256] = np.asarray(inputs["e2_w"], np.float16)
    wblob[:, W_E30:W_E30 + 128] = np.asarray(inputs["e3_w"], np.float16)[0:128]
    wblob[:, W_E31:W_E31 + 128] = np.asarray(inputs["e3_w"], np.float16)[128:256]
    wblob[:, W_BR:W_BR + 128] = br.astype(np.float16)
    wblob[:, W_B2:W_B2 + 128] = b2
    wblob[0:NODE_DIM, W_P1:W_P1 + 128] = np.asarray(inputs["p1_w"], np.float16)
    wblob[NODE_DIM, W_P1:W_P1 + 128] = np.asarray(inputs["p1_b"], np.float16)
    wblob[0:EDGE_DIM, W_E1:W_E1 + 128] = np.asarray(inputs["e1_w"], np.float16)

    rows16 = np.zeros((1, 512), np.float16)
    rows16[0, :] = np.tile(np.asarray(inputs["e3_b"], np.float16), 4)

    bias_c = np.zeros((128, 4), np.float32)
    bias_c[:, B_E1] = np.asarray(inputs["e1_b"], np.float32)
    bias_c[:, B_E2:B_E2 + 2] = np.asarray(
        inputs["e2_b"], np.float32).reshape(2, 128).T

    com = {"wblob": wblob, "rows16": rows16, "bias32": bias_c, "a2h": a2h}
    com = {k: np.ascontiguousarray(v) for k, v in com.items()}

    in_maps = []
    wg_all = np.zeros((N_CORES, G_PER_CORE), np.float32)
    for c in range(N_CORES):
        ea_s = np.zeros((EP, EDGE_DIM), np.float32)
        xs_s = np.zeros((EP, XD), np.float32)
        w_s = np.zeros(EP, np.float32)
        for s in range(G_PER_CORE):
            es = np.where(gid == c * G_PER_CORE + s)[0]
            assert len(es) <= CAP, f"graph {c * G_PER_CORE + s}: {len(es)} edges"
            pos = s * CAP + np.arange(len(es))
            we = w_all[dst[es]]
            ea_s[pos] = ea[es]
            xs_s[pos, 0:NODE_DIM] = x[src[es]] * we[:, None]
            xs_s[pos, NODE_DIM] = we
            w_s[pos] = we
            wg_all[c, s] = we.sum()

        edge16 = np.zeros((EDGE_DIM + XD, EP), np.float16)
        edge16[0:EDGE_DIM] = ea_s.T
        edge16[EDGE_DIM:EDGE_DIM + XD] = xs_s.T

        m = dict(com)
        m["edge16"] = np.ascontiguousarray(edge16)
        m["wme"] = np.ascontiguousarray(w_s.reshape(NT, 128).T.astype(np.float16))
        in_maps.append(m)
    return in_maps, wg_all, br2


def _run(inputs, trace=False, tmpdir=None):
    nc = _build()
    in_maps, wg_all, br2 = _host_prep(inputs)
    if trace:
        # No egress in this sandbox: neutralize the artifact upload the
        # trace path performs after NTFF capture, and register the NTFF
        # hook module if the image's antenv package lacks axon_hooks.
        from concourse import bass_utils as _bu
        _bu.upload_artifacts = lambda d: d
        try:
            from antenv import axon_hooks  # noqa: F401
        except ImportError:
            import importlib.util, sys as _sys
            spec = importlib.util.spec_from_file_location(
                "antenv.axon_hooks", "/opt/trn_rl_repo/antenv/axon_hooks.py")
            mod = importlib.util.module_from_spec(spec)
            spec.loader.exec_module(mod)
            _sys.modules["antenv.axon_hooks"] = mod
    res = run_bass_kernel_spmd(nc, in_maps, list(range(N_CORES)),
                               trace=trace, tmpdir=tmpdir)
    out = np.empty((G, H), np.float32)
    for c in range(N_CORES):
        p = res.results[c]["pooled"]
        acc = np.zeros((G_PER_CORE, H), np.float32)
        for j in range(4):
            acc += p[32 * j:32 * j + G_PER_CORE]
            acc += p[32 * j + G_PER_CORE:32 * j + 2 * G_PER_CORE]
        acc += wg_all[c][:, None] * br2[None, :]   # W_g * (p2b @ Br)
        out[c * G_PER_CORE:(c + 1) * G_PER_CORE, :] = acc
    return out, res


def kernel(**inputs) -> np.ndarray:
    out, _ = _run(inputs)
    return out


# revision 29
# speedup vs baseline: 1.3287x; 1.0229x over previous
"""Trainium2 Bass kernel for nn_Net_16174846837292 (NNConv GNN message passing).

Strategy (graph-sharded, aggregation-folded, single fp16 a2 pass):
  pooled[g,o] = sum_{e: batch[dst[e]]=g} w_e * msg[e,o],  w_e = 1/max(cnt[dst_e],1)
  msg[e,o]    = sum_{k,i} e3[e,k]*h[src_e,i]*e4w[k,i*128+o] + sum_i h[src_e,i]*e4b[i*128+o]
  => pooled[g,o] = sum_k ZG_g[:,k]^T A2f[:,k*128+o] + HW_g^T Br
     ZG_g[i,k] = sum_{e in g} (w_e h[src_e,i]) e3[e,k],  HW_g[i] = sum_e w_e h[src_e,i]

Sharding: edges grouped by the graph of their destination node; 8 graphs per
core, so in-degree weights are per-edge host constants and NO collectives are
needed. Per-core edges pack into 8 slots of 192 (64-aligned segments).

Device pipeline (all PE operands 16-bit; fp32 would cost two array passes):
- host pre-gathers x[src_e], scales it by w_e and appends a w_e row, matched
  against [p1w; p1b]: the node MLP needs no bias handling and no per-tile
  w_e scaling on the device at all.
- last layer of each MLP runs edge-major (previous activations as the matmul
  stationary operand): no transposes, no gathers anywhere.
- ZG accumulates per graph slot in PSUM ([ZG | HW] via a ones column), then
  splits into bf16 hi/lo halves [zh|zl] (bf16 = fp32 exponent range, no
  subnormal trouble).
- the 4 MB fp16 a2 streams once as the moving operand against the 16-col
  [zh|zl] stationaries, col-tiled 4-wide across the PE array; the e4-bias
  term rides the same accumulation as hw_f @ Br.
- big a2 DMAs are gated behind the small input loads (a big transfer issued
  early monopolizes the 16 SDMA engines and stalls every input behind it),
  and tiny warmup matmuls keep the PE HAM clock at 2.4 GHz while they land.

Measured error vs the fp32 reference: ~4e-4 of output scale.
"""

import numpy as np
from contextlib import ExitStack

import concourse.bass as bass
import concourse.tile as tile
from concourse import bacc, mybir
from concourse.bass_utils import run_bass_kernel_spmd

N_CORES = 8
N, E, G, H = 4096, 8192, 64, 128
NODE_DIM, EDGE_DIM = 11, 5
G_PER_CORE = G // N_CORES          # 8 graph slots per core
CAP = 192                          # edge slots per graph (64-aligned segments)
EP = G_PER_CORE * CAP              # 1536 edge slots per core
NT = EP // 128                     # 12 edge tiles per core
NCH = EP // 512                    # 3 512-wide chunks for the feature-major MLPs
XD = NODE_DIM + 1                  # xs rows: w*x (11) + w (1)

f32 = mybir.dt.float32
f16 = mybir.dt.float16
bf16 = mybir.dt.bfloat16
AF = mybir.ActivationFunctionType
OP = mybir.AluOpType

# wblob column map (f16 weights packed into one [128, 1024] blob)
W_P2, W_E2, W_E30, W_E31, W_BR, W_P1, W_E1, W_B2 = (
    0, 128, 384, 512, 640, 768, 896, 1024)
WBW = 1152
# bias32 column map ([128, 4] f32)
B_E1, B_E2 = 0, 1


def _slot_segments(s):
    """(tile, p0, p1) segments of graph slot s in the (p, t) edge grid."""
    segs, a, end = [], s * CAP, (s + 1) * CAP
    while a < end:
        t, p0 = divmod(a, 128)
        take = min(128 - p0, end - a)
        segs.append((t, p0, p0 + take))
        a += take
    return segs


def _emit(nc, tc, io):
    es = ExitStack()
    const = es.enter_context(tc.tile_pool(name="const", bufs=1))
    a2pool = es.enter_context(tc.tile_pool(name="a2pool", bufs=1))
    big = es.enter_context(tc.tile_pool(name="big", bufs=1))
    work = es.enter_context(tc.tile_pool(name="work", bufs=4))
    psA = es.enter_context(tc.tile_pool(name="psA", bufs=3, space="PSUM"))
    psB = es.enter_context(tc.tile_pool(name="psB", bufs=2, space="PSUM"))
    psZ = es.enter_context(tc.tile_pool(name="psZ", bufs=2, space="PSUM"))
    psO = es.enter_context(tc.tile_pool(name="psO", bufs=1, space="PSUM"))

    with es:
        a2 = a2pool.tile([128, 128 * H], f16, tag="a2")

        wblob = const.tile([128, WBW], f16, tag="wblob")
        nc.sync.dma_start(wblob[:], io["wblob"][:, :])
        eaT = const.tile([EDGE_DIM, EP], f16, tag="eaT")
        nc.sync.dma_start(eaT[:], io["edge16"][0:EDGE_DIM, :])
        xsT = const.tile([XD, EP], f16, tag="xsT")
        nc.sync.dma_start(xsT[:], io["edge16"][EDGE_DIM:EDGE_DIM + XD, :])
        wme = const.tile([128, NT], f16, tag="wme")
        nc.sync.dma_start(wme[:], io["wme"][:, :])
        bias32 = const.tile([128, 4], f32, tag="bias32")
        nc.sync.dma_start(bias32[:], io["bias32"][:, :])
        rows16 = const.tile([1, 512], f16, tag="rows16")
        last_dma = nc.sync.dma_start(rows16[:], io["rows16"][:, :])

        # a2 halves on the idle sync/gpsimd queues, gated behind the last
        # small input load: a big transfer issued early monopolizes the 16
        # SDMA engines and delays every small input DMA behind it
        KSYNC = 72 * 128
        a2d0 = nc.sync.dma_start(a2[:, 0:KSYNC], io["a2h"][:, 0:KSYNC])
        a2d1 = nc.gpsimd.dma_start(a2[:, KSYNC:128 * H], io["a2h"][:, KSYNC:128 * H])
        for ad in (a2d0, a2d1):
            bass._add_dep_helper(ad.ins, last_dma.ins, sync=True,
                                 reason="a2 after small input DMAs")

        # spin tiny matmuls while input DMAs land so HAM is warm (2.4 GHz)
        # when the real pipeline starts
        ones_r = const.tile([1, 128], f16, tag="ones_r")
        nc.vector.memset(ones_r[:], 1.0)
        for _ in range(40):
            pw = psA.tile([128, 512], f32, tag="mlp")
            nc.tensor.matmul(pw[0:16, 0:16], ones_r[:, 0:16], ones_r[:, 0:16],
                             start=True, stop=True)

        # ---- feature-major MLP interiors (epilogues split ACT/DVE) ----------
        relu1 = big.tile([128, EP], f16, tag="relu1")
        e1o = big.tile([128, EP], f16, tag="e1o")
        e2o0 = big.tile([128, EP], f16, tag="e2o0")
        e2o1 = big.tile([128, EP], f16, tag="e2o1")
        for q in range(NCH):
            sl = slice(q * 512, (q + 1) * 512)
            ps = psA.tile([128, 512], f32, tag="mlp")
            nc.tensor.matmul(ps[:], wblob[0:EDGE_DIM, W_E1:W_E1 + 128],
                             eaT[:, sl], start=True, stop=True)
            nc.scalar.activation(e1o[:, sl], ps[:], AF.Relu,
                                 bias=bias32[:, B_E1:B_E1 + 1])
            ps2 = psA.tile([128, 512], f32, tag="mlp")
            nc.tensor.matmul(ps2[:], wblob[0:XD, W_P1:W_P1 + 128],
                             xsT[:, sl], start=True, stop=True)
            nc.vector.tensor_scalar_max(relu1[:, sl], ps2[:], 0.0)
        for q in range(NCH):
            for m, e2o in enumerate((e2o0, e2o1)):
                sl = slice(q * 512, (q + 1) * 512)
                ps = psA.tile([128, 512], f32, tag="mlp")
                nc.tensor.matmul(ps[:], wblob[:, W_E2 + m * 128:W_E2 + (m + 1) * 128],
                                 e1o[:, sl], start=True, stop=True)
                if m == 0:
                    nc.scalar.activation(e2o[:, sl], ps[:], AF.Relu,
                                         bias=bias32[:, B_E2 + m:B_E2 + m + 1])
                else:
                    nc.vector.tensor_scalar(e2o[:, sl], ps[:],
                                            bias32[:, B_E2 + m:B_E2 + m + 1],
                                            0.0, op0=OP.add, op1=OP.max)

        # broadcast e3b (tiled 4x) to all partitions for 512-wide adds
        pbc2 = psA.tile([128, 512], f32, tag="mlp")
        nc.tensor.matmul(pbc2[:], ones_r[:], rows16[0:1, 0:512],
                         start=True, stop=True)
        e3bb = const.tile([128, 512], f32, tag="e3bb")
        nc.scalar.copy(e3bb[:], pbc2[:])

        # ---- edge-major last layers, 4-tile groups, 512-wide epilogues ------
        # h_big[e, t, i] = relu1[:, e].T @ p2w + p2b  (w_e, p1b folded on host)
        # e3x[e, t, k]   = relu(e2o[:, e].T @ e3w + e3b); col H = 1.0
        h_big = big.tile([128, NT, H], f16, tag="hbig")
        e3x = big.tile([128, NT, H + 1], f16, tag="e3x")
        nc.gpsimd.memset(e3x[:, :, H:H + 1], 1.0)
        for g4 in range(NT // 4):
            psh = psB.tile([128, 512], f32, tag="he4")
            pse = psA.tile([128, 512], f32, tag="mlp")
            for j in range(4):
                t = g4 * 4 + j
                sl = slice(t * 128, (t + 1) * 128)
                jj = slice(j * 128, (j + 1) * 128)
                nc.tensor.matmul(psh[:, jj], relu1[:, sl], wblob[:, W_P2:W_P2 + 128],
                                 start=True, stop=True)
                nc.tensor.matmul(pse[:, jj], e2o0[:, sl], wblob[:, W_E30:W_E30 + 128],
                                 start=True, stop=False)
                nc.tensor.matmul(pse[:, jj], e2o1[:, sl], wblob[:, W_E31:W_E31 + 128],
                                 start=False, stop=True)
            g4s = slice(4 * g4, 4 * g4 + 4)
            if g4 % 2 == 0:
                nc.scalar.copy(h_big[:, g4s, :], psh[:])
            else:
                nc.vector.tensor_copy(h_big[:, g4s, :], psh[:])
            t4 = work.tile([128, 512], f32, tag="t4")
            nc.vector.tensor_tensor(t4[:], pse[:], e3bb[:], op=OP.add)
            nc.scalar.activation(e3x[:, g4s, 0:H], t4[:], AF.Relu)

        # ---- per-graph ZG accumulation + bf16 hi/lo split --------------------
        # pz cols 0:128 = ZG (stationary h), col 128 = HW (ones col of e3x)
        zg2 = big.tile([128, 2 * G_PER_CORE, H], bf16, tag="zg2")
        hw_f = work.tile([128, 2 * G_PER_CORE], f16, tag="hwf")
        nc.vector.memset(hw_f[:, G_PER_CORE:2 * G_PER_CORE], 0.0)
        s_f = work.tile([128, G_PER_CORE], f16, tag="sf")
        for s in range(G_PER_CORE):
            segs = _slot_segments(s)
            pz = psZ.tile([128, H + 2], f32, tag="zg")
            # the S chain starts only after the ZG chain closes: a matmul
            # with start=True clears has_written for its partitions across
            # the whole PSUM region, killing any open chain there
            for n, (t, p0, p1) in enumerate(segs):
                nc.tensor.matmul(pz[:, 0:H + 1], h_big[p0:p1, t, :],
                                 e3x[p0:p1, t, :],
                                 start=(n == 0), stop=(n == len(segs) - 1))
            for n, (t, p0, p1) in enumerate(segs):
                nc.tensor.matmul(pz[:, H + 1:H + 2], e3x[p0:p1, t, 0:H],
                                 wme[p0:p1, t:t + 1],
                                 start=(n == 0), stop=(n == len(segs) - 1))
            if s % 2 == 0:
                nc.scalar.copy(zg2[:, s, :], pz[:, 0:H])
            else:
                nc.vector.tensor_copy(zg2[:, s, :], pz[:, 0:H])
            nc.vector.tensor_tensor(zg2[:, G_PER_CORE + s, :], pz[:, 0:H],
                                    zg2[:, s, :], op=OP.subtract)
            nc.vector.tensor_copy(hw_f[:, s:s + 1], pz[:, H:H + 1])
            nc.vector.tensor_copy(s_f[:, s:s + 1], pz[:, H + 1:H + 2])

        # ---- final a2 contraction: a2 streams as the moving operand ----------
        # the e4-bias term (hw_f @ Br) rides as the last accumulation step of
        # col-group 0 (hw_f cols 8:16 are zeros)
        ot = work.tile([128, 128], f32, tag="ot")
        nc.gpsimd.memset(ot[:], 0.0)
        po = psO.tile([128, 128], f32, tag="out")
        for k4 in range(H // 4):
            for j in range(4):
                k = k4 * 4 + j
                last = (k4 == H // 4 - 1) and (j != 0)
                nc.tensor.matmul(po[32 * j:32 * j + 16, :], zg2[:, :, k],
                                 a2[:, k * 128:(k + 1) * 128],
                                 start=(k4 == 0), stop=last,
                                 tile_position=(0, 32 * j))
        nc.tensor.matmul(po[0:16, :], hw_f[:], wblob[:, W_BR:W_BR + 128],
                         start=False, stop=False, tile_position=(0, 0))
        nc.tensor.matmul(po[0:G_PER_CORE, :], s_f[:], wblob[:, W_B2:W_B2 + 128],
                         start=False, stop=True, tile_position=(0, 0))
        for j in range(4):
            if j % 2 == 0:
                nc.scalar.copy(ot[32 * j:32 * j + 16, :], po[32 * j:32 * j + 16, :])
            else:
                nc.vector.tensor_copy(ot[32 * j:32 * j + 16, :],
                                      po[32 * j:32 * j + 16, :])
        nc.sync.dma_start(io["pooled"][0:64, :], ot[0:64, :])
        nc.sync.dma_start(io["pooled"][64:128, :], ot[64:128, :])


_CACHE = {}


def _build():
    if "nc" in _CACHE:
        return _CACHE["nc"]
    nc = bacc.Bacc("TRN2", target_bir_lowering=False, debug=False,
                   num_devices=N_CORES)
    io = {}

    def din(name, shape, dt=f32):
        io[name] = nc.dram_tensor(name, shape, dt, kind="ExternalInput").ap()

    din("edge16", [EDGE_DIM + XD, EP], f16)
    din("wblob", [128, WBW], f16)
    din("wme", [128, NT], f16)
    din("bias32", [128, 4])
    din("rows16", [1, 512], f16)
    din("a2h", [128, 128 * H], f16)
    io["pooled"] = nc.dram_tensor("pooled", [128, H], f32,
                                  kind="ExternalOutput").ap()

    with tile.TileContext(nc) as tc:
        _emit(nc, tc, io)
    nc.compile()
    _CACHE["nc"] = nc
    return nc


def _host_prep(inputs):
    x = np.asarray(inputs["x"], dtype=np.float32)
    ea = np.asarray(inputs["edge_attr"], dtype=np.float32)
    ei = np.asarray(inputs["edge_index"]).astype(np.int64)
    batch = np.asarray(inputs["batch"]).astype(np.int64)
    src, dst = ei[0], ei[1]
    gid = batch[dst]
    cnt = np.bincount(dst, minlength=N).astype(np.float32)
    w_all = 1.0 / np.maximum(cnt, 1.0)

    e4w = np.asarray(inputs["e4_w"], np.float32).reshape(128, 128, 128)
    a2h = np.ascontiguousarray(
        e4w.transpose(1, 0, 2).reshape(128, 128 * H).astype(np.float16))
    p2b = np.asarray(inputs["p2_b"], np.float32)
    b2 = np.einsum("i,kio->ko", p2b, e4w).astype(np.float16)   # [k, o]
    br = np.asarray(inputs["e4_b"], np.float32).reshape(128, 128)
    br2 = p2b @ br                                             # [o]

    wblob = np.zeros((128, WBW), np.float16)
    wblob[:, W_P2:W_P2 + 128] = np.asarray(inputs["p2_w"], np.float16)
    wblob[:, W_E2:W_E2 + 256] = np.asarray(inputs["e2_w"], np.float16)
    wblob[:, W_E30:W_E30 + 128] = np.asarray(inputs["e3_w"], np.float16)[0:128]
    wblob[:, W_E31:W_E31 + 128] = np.asarray(inputs["e3_w"], np.float16)[128:256]
    wblob[:, W_BR:W_BR + 128] = br.astype(np.float16)
    wblob[:, W_B2:W_B2 + 128] = b2
    wblob[0:NODE_DIM, W_P1:W_P1 + 128] = np.asarray(inputs["p1_w"], np.float16)
    wblob[NODE_DIM, W_P1:W_P1 + 128] = np.asarray(inputs["p1_b"], np.float16)
    wblob[0:EDGE_DIM, W_E1:W_E1 + 128] = np.asarray(inputs["e1_w"], np.float16)

    rows16 = np.zeros((1, 512), np.float16)
    rows16[0, :] = np.tile(np.asarray(inputs["e3_b"], np.float16), 4)

    bias_c = np.zeros((128, 4), np.float32)
    bias_c[:, B_E1] = np.asarray(inputs["e1_b"], np.float32)
    bias_c[:, B_E2:B_E2 + 2] = np.asarray(
        inputs["e2_b"], np.float32).reshape(2, 128).T

    com = {"wblob": wblob, "rows16": rows16, "bias32": bias_c, "a2h": a2h}
    com = {k: np.ascontiguousarray(v) for k, v in com.items()}

    in_maps = []
    wg_all = np.zeros((N_CORES, G_PER_CORE), np.float32)
    for c in range(N_CORES):
        ea_s = np.zeros((EP, EDGE_DIM), np.float32)
        xs_s = np.zeros((EP, XD), np.float32)
        w_s = np.zeros(EP, np.float32)
        for s in range(G_PER_CORE):
            es = np.where(gid == c * G_PER_CORE + s)[0]
            assert len(es) <= CAP, f"graph {c * G_PER_CORE + s}: {len(es)} edges"
            pos = s * CAP + np.arange(len(es))
            we = w_all[dst[es]]
            ea_s[pos] = ea[es]
            xs_s[pos, 0:NODE_DIM] = x[src[es]] * we[:, None]
            xs_s[pos, NODE_DIM] = we
            w_s[pos] = we
            wg_all[c, s] = we.sum()

        edge16 = np.zeros((EDGE_DIM + XD, EP), np.float16)
        edge16[0:EDGE_DIM] = ea_s.T
        edge16[EDGE_DIM:EDGE_DIM + XD] = xs_s.T

        m = dict(com)
        m["edge16"] = np.ascontiguousarray(edge16)
        m["wme"] = np.ascontiguousarray(w_s.reshape(NT, 128).T.astype(np.float16))
        in_maps.append(m)
    return in_maps, wg_all, br2


def _run(inputs, trace=False, tmpdir=None):
    nc = _build()
    in_maps, wg_all, br2 = _host_prep(inputs)
    if trace:
        # No egress in this sandbox: neutralize the artifact upload the
        # trace path performs after NTFF capture, and register the NTFF
        # hook module if the image's antenv package lacks axon_hooks.
        from concourse import bass_utils as _bu
        _bu.upload_artifacts = lambda d: d
        try:
            from antenv import axon_hooks  # noqa: F401
        except ImportError:
            import importlib.util, sys as _sys
            spec = importlib.util.spec_from_file_location(
                "antenv.axon_hooks", "/opt/trn_rl_repo/antenv/axon_hooks.py")
            mod = importlib.util.module_from_spec(spec)
            spec.loader.exec_module(mod)
            _sys.modules["antenv.axon_hooks"] = mod
    res = run_bass_kernel_spmd(nc, in_maps, list(range(N_CORES)),
                               trace=trace, tmpdir=tmpdir)
    out = np.empty((G, H), np.float32)
    for c in range(N_CORES):
        p = res.results[c]["pooled"]
        acc = np.zeros((G_PER_CORE, H), np.float32)
        for j in range(4):
            acc += p[32 * j:32 * j + G_PER_CORE]
            acc += p[32 * j + G_PER_CORE:32 * j + 2 * G_PER_CORE]
        acc += wg_all[c][:, None] * br2[None, :]   # W_g * (p2b @ Br)
        out[c * G_PER_CORE:(c + 1) * G_PER_CORE, :] = acc
    return out, res


def kernel(**inputs) -> np.ndarray:
    out, _ = _run(inputs)
    return out


# revision 31
# speedup vs baseline: 1.3705x; 1.0314x over previous
"""Trainium2 Bass kernel for nn_Net_16174846837292 (NNConv GNN message passing).

Strategy (graph-sharded, aggregation-folded, single fp16 a2 pass):
  pooled[g,o] = sum_{e: batch[dst[e]]=g} w_e * msg[e,o],  w_e = 1/max(cnt[dst_e],1)
  msg[e,o]    = sum_{k,i} e3[e,k]*h[src_e,i]*e4w[k,i*128+o] + sum_i h[src_e,i]*e4b[i*128+o]
  => pooled[g,o] = sum_k ZG_g[:,k]^T A2f[:,k*128+o] + HW_g^T Br
     ZG_g[i,k] = sum_{e in g} (w_e h[src_e,i]) e3[e,k],  HW_g[i] = sum_e w_e h[src_e,i]

Sharding: edges grouped by the graph of their destination node; 8 graphs per
core, so in-degree weights are per-edge host constants and NO collectives are
needed. Per-core edges pack into 8 slots of 192 (64-aligned segments).

Device pipeline (all PE operands 16-bit; fp32 would cost two array passes):
- host pre-gathers x[src_e], scales it by w_e and appends a w_e row, matched
  against [p1w; p1b]: the node MLP needs no bias handling and no per-tile
  w_e scaling on the device at all.
- last layer of each MLP runs edge-major (previous activations as the matmul
  stationary operand): no transposes, no gathers anywhere.
- ZG accumulates per graph slot in PSUM ([ZG | HW] via a ones column), then
  splits into bf16 hi/lo halves [zh|zl] (bf16 = fp32 exponent range, no
  subnormal trouble).
- the 4 MB fp16 a2 streams once as the moving operand against the 16-col
  [zh|zl] stationaries, col-tiled 4-wide across the PE array; the e4-bias
  term rides the same accumulation as hw_f @ Br.
- big a2 DMAs are gated behind the small input loads (a big transfer issued
  early monopolizes the 16 SDMA engines and stalls every input behind it),
  and tiny warmup matmuls keep the PE HAM clock at 2.4 GHz while they land.

Measured error vs the fp32 reference: ~4e-4 of output scale.
"""

import numpy as np
from contextlib import ExitStack

import concourse.bass as bass
import concourse.tile as tile
from concourse import bacc, mybir
from concourse.bass_utils import run_bass_kernel_spmd

N_CORES = 8
N, E, G, H = 4096, 8192, 64, 128
NODE_DIM, EDGE_DIM = 11, 5
G_PER_CORE = G // N_CORES          # 8 graph slots per core
CAP = 192                          # edge slots per graph (64-aligned segments)
EP = G_PER_CORE * CAP              # 1536 edge slots per core
NT = EP // 128                     # 12 edge tiles per core
NCH = EP // 512                    # 3 512-wide chunks for the feature-major MLPs
XD = NODE_DIM + 1                  # xs rows: w*x (11) + w (1)

f32 = mybir.dt.float32
f16 = mybir.dt.float16
bf16 = mybir.dt.bfloat16
AF = mybir.ActivationFunctionType
OP = mybir.AluOpType

# wblob column map (f16 weights packed into one [128, 1024] blob)
W_P2, W_E2, W_E30, W_E31, W_BR, W_P1, W_E1, W_B2 = (
    0, 128, 384, 512, 640, 768, 896, 1024)
WBW = 1152
# bias32 column map ([128, 4] f32)
B_E1, B_E2 = 0, 1


def _slot_segments(s):
    """(tile, p0, p1) segments of graph slot s in the (p, t) edge grid."""
    segs, a, end = [], s * CAP, (s + 1) * CAP
    while a < end:
        t, p0 = divmod(a, 128)
        take = min(128 - p0, end - a)
        segs.append((t, p0, p0 + take))
        a += take
    return segs


def _emit(nc, tc, io):
    es = ExitStack()
    const = es.enter_context(tc.tile_pool(name="const", bufs=1))
    a2pool = es.enter_context(tc.tile_pool(name="a2pool", bufs=1))
    big = es.enter_context(tc.tile_pool(name="big", bufs=1))
    work = es.enter_context(tc.tile_pool(name="work", bufs=4))
    psA = es.enter_context(tc.tile_pool(name="psA", bufs=3, space="PSUM"))
    psB = es.enter_context(tc.tile_pool(name="psB", bufs=2, space="PSUM"))
    psZ = es.enter_context(tc.tile_pool(name="psZ", bufs=2, space="PSUM"))
    psO = es.enter_context(tc.tile_pool(name="psO", bufs=1, space="PSUM"))

    with es:
        a2 = a2pool.tile([128, 128 * H], f16, tag="a2")

        wblob = const.tile([128, WBW], f16, tag="wblob")
        nc.sync.dma_start(wblob[:], io["wblob"][:, :])
        eaT = const.tile([EDGE_DIM, EP], f16, tag="eaT")
        nc.sync.dma_start(eaT[:], io["edge16"][0:EDGE_DIM, :])
        xsT = const.tile([XD, EP], f16, tag="xsT")
        last_sync = nc.sync.dma_start(xsT[:], io["edge16"][EDGE_DIM:EDGE_DIM + XD, :])
        wme = const.tile([128, NT], f16, tag="wme")
        nc.gpsimd.dma_start(wme[:], io["wme"][:, :])
        bias32 = const.tile([128, 4], f32, tag="bias32")
        nc.gpsimd.dma_start(bias32[:], io["bias32"][:, :])
        rows16 = const.tile([1, 512], f16, tag="rows16")
        last_dma = nc.gpsimd.dma_start(rows16[:], io["rows16"][:, :])

        # a2 halves on the idle sync/gpsimd queues, gated behind the last
        # small input load: a big transfer issued early monopolizes the 16
        # SDMA engines and delays every small input DMA behind it
        KSYNC = 72 * 128
        a2d0 = nc.sync.dma_start(a2[:, 0:KSYNC], io["a2h"][:, 0:KSYNC])
        a2d1 = nc.gpsimd.dma_start(a2[:, KSYNC:128 * H], io["a2h"][:, KSYNC:128 * H])
        for ad in (a2d0, a2d1):
            for dep in (last_dma, last_sync):
                bass._add_dep_helper(ad.ins, dep.ins, sync=True,
                                     reason="a2 after small input DMAs")

        # spin tiny matmuls while input DMAs land so HAM is warm (2.4 GHz)
        # when the real pipeline starts
        ones_r = const.tile([1, 128], f16, tag="ones_r")
        nc.vector.memset(ones_r[:], 1.0)
        for _ in range(40):
            pw = psA.tile([128, 512], f32, tag="mlp")
            nc.tensor.matmul(pw[0:16, 0:16], ones_r[:, 0:16], ones_r[:, 0:16],
                             start=True, stop=True)

        # ---- feature-major MLP interiors (epilogues split ACT/DVE) ----------
        relu1 = big.tile([128, EP], f16, tag="relu1")
        e1o = big.tile([128, EP], f16, tag="e1o")
        e2o0 = big.tile([128, EP], f16, tag="e2o0")
        e2o1 = big.tile([128, EP], f16, tag="e2o1")
        for q in range(NCH):
            sl = slice(q * 512, (q + 1) * 512)
            ps = psA.tile([128, 512], f32, tag="mlp")
            nc.tensor.matmul(ps[:], wblob[0:EDGE_DIM, W_E1:W_E1 + 128],
                             eaT[:, sl], start=True, stop=True)
            nc.scalar.activation(e1o[:, sl], ps[:], AF.Relu,
                                 bias=bias32[:, B_E1:B_E1 + 1])
            ps2 = psA.tile([128, 512], f32, tag="mlp")
            nc.tensor.matmul(ps2[:], wblob[0:XD, W_P1:W_P1 + 128],
                             xsT[:, sl], start=True, stop=True)
            nc.vector.tensor_scalar_max(relu1[:, sl], ps2[:], 0.0)
        for q in range(NCH):
            for m, e2o in enumerate((e2o0, e2o1)):
                sl = slice(q * 512, (q + 1) * 512)
                ps = psA.tile([128, 512], f32, tag="mlp")
                nc.tensor.matmul(ps[:], wblob[:, W_E2 + m * 128:W_E2 + (m + 1) * 128],
                                 e1o[:, sl], start=True, stop=True)
                if m == 0:
                    nc.scalar.activation(e2o[:, sl], ps[:], AF.Relu,
                                         bias=bias32[:, B_E2 + m:B_E2 + m + 1])
                else:
                    nc.vector.tensor_scalar(e2o[:, sl], ps[:],
                                            bias32[:, B_E2 + m:B_E2 + m + 1],
                                            0.0, op0=OP.add, op1=OP.max)

        # broadcast e3b (tiled 4x) to all partitions for 512-wide adds
        pbc2 = psA.tile([128, 512], f32, tag="mlp")
        nc.tensor.matmul(pbc2[:], ones_r[:], rows16[0:1, 0:512],
                         start=True, stop=True)
        e3bb = const.tile([128, 512], f32, tag="e3bb")
        nc.scalar.copy(e3bb[:], pbc2[:])

        # ---- edge-major last layers, 4-tile groups, 512-wide epilogues ------
        # h_big[e, t, i] = relu1[:, e].T @ p2w + p2b  (w_e, p1b folded on host)
        # e3x[e, t, k]   = relu(e2o[:, e].T @ e3w + e3b); col H = 1.0
        h_big = big.tile([128, NT, H], f16, tag="hbig")
        e3x = big.tile([128, NT, H + 1], f16, tag="e3x")
        nc.gpsimd.memset(e3x[:, :, H:H + 1], 1.0)
        for g4 in range(NT // 4):
            psh = psB.tile([128, 512], f32, tag="he4")
            pse = psA.tile([128, 512], f32, tag="mlp")
            for j in range(4):
                t = g4 * 4 + j
                sl = slice(t * 128, (t + 1) * 128)
                jj = slice(j * 128, (j + 1) * 128)
                nc.tensor.matmul(psh[:, jj], relu1[:, sl], wblob[:, W_P2:W_P2 + 128],
                                 start=True, stop=True)
                nc.tensor.matmul(pse[:, jj], e2o0[:, sl], wblob[:, W_E30:W_E30 + 128],
                                 start=True, stop=False)
                nc.tensor.matmul(pse[:, jj], e2o1[:, sl], wblob[:, W_E31:W_E31 + 128],
                                 start=False, stop=True)
            g4s = slice(4 * g4, 4 * g4 + 4)
            if g4 % 2 == 0:
                nc.scalar.copy(h_big[:, g4s, :], psh[:])
            else:
                nc.vector.tensor_copy(h_big[:, g4s, :], psh[:])
            t4 = work.tile([128, 512], f32, tag="t4")
            nc.vector.tensor_tensor(t4[:], pse[:], e3bb[:], op=OP.add)
            nc.scalar.activation(e3x[:, g4s, 0:H], t4[:], AF.Relu)

        # ---- per-graph ZG accumulation + bf16 hi/lo split --------------------
        # pz cols 0:128 = ZG (stationary h), col 128 = HW (ones col of e3x)
        zg2 = big.tile([128, 2 * G_PER_CORE, H], bf16, tag="zg2")
        hw_f = work.tile([128, 2 * G_PER_CORE], f16, tag="hwf")
        nc.vector.memset(hw_f[:, G_PER_CORE:2 * G_PER_CORE], 0.0)
        s_f = work.tile([128, G_PER_CORE], f16, tag="sf")
        for s in range(G_PER_CORE):
            segs = _slot_segments(s)
            pz = psZ.tile([128, H + 2], f32, tag="zg")
            # the S chain starts only after the ZG chain closes: a matmul
            # with start=True clears has_written for its partitions across
            # the whole PSUM region, killing any open chain there
            for n, (t, p0, p1) in enumerate(segs):
                nc.tensor.matmul(pz[:, 0:H + 1], h_big[p0:p1, t, :],
                                 e3x[p0:p1, t, :],
                                 start=(n == 0), stop=(n == len(segs) - 1))
            for n, (t, p0, p1) in enumerate(segs):
                nc.tensor.matmul(pz[:, H + 1:H + 2], e3x[p0:p1, t, 0:H],
                                 wme[p0:p1, t:t + 1],
                                 start=(n == 0), stop=(n == len(segs) - 1))
            if s % 2 == 0:
                nc.scalar.copy(zg2[:, s, :], pz[:, 0:H])
            else:
                nc.vector.tensor_copy(zg2[:, s, :], pz[:, 0:H])
            nc.vector.tensor_tensor(zg2[:, G_PER_CORE + s, :], pz[:, 0:H],
                                    zg2[:, s, :], op=OP.subtract)
            nc.vector.tensor_copy(hw_f[:, s:s + 1], pz[:, H:H + 1])
            nc.vector.tensor_copy(s_f[:, s:s + 1], pz[:, H + 1:H + 2])

        # ---- final a2 contraction: a2 streams as the moving operand ----------
        # the e4-bias term (hw_f @ Br) rides as the last accumulation step of
        # col-group 0 (hw_f cols 8:16 are zeros)
        ot = work.tile([128, 128], f32, tag="ot")
        nc.gpsimd.memset(ot[:], 0.0)
        po = psO.tile([128, 128], f32, tag="out")
        # groups 1-3 take 30 k's each, group 0 takes 38 + the bias terms:
        # early groups free their output rows while group 0 still streams
        KL = [list(range(0, 8)) + list(range(98, 128)),
              list(range(8, 38)), list(range(38, 68)), list(range(68, 98))]
        for r in range(38):
            for j in range(4):
                if r >= len(KL[j]):
                    continue
                k = KL[j][r]
                nc.tensor.matmul(po[32 * j:32 * j + 16, :], zg2[:, :, k],
                                 a2[:, k * 128:(k + 1) * 128],
                                 start=(r == 0),
                                 stop=(j != 0 and r == len(KL[j]) - 1),
                                 tile_position=(0, 32 * j))
        nc.tensor.matmul(po[0:16, :], hw_f[:], wblob[:, W_BR:W_BR + 128],
                         start=False, stop=False, tile_position=(0, 0))
        nc.tensor.matmul(po[0:G_PER_CORE, :], s_f[:], wblob[:, W_B2:W_B2 + 128],
                         start=False, stop=True, tile_position=(0, 0))
        for j in (1, 2, 3, 0):
            if j % 2 == 0:
                nc.scalar.copy(ot[32 * j:32 * j + 16, :], po[32 * j:32 * j + 16, :])
            else:
                nc.vector.tensor_copy(ot[32 * j:32 * j + 16, :],
                                      po[32 * j:32 * j + 16, :])
        nc.sync.dma_start(io["pooled"][32:128, :], ot[32:128, :])
        nc.sync.dma_start(io["pooled"][0:32, :], ot[0:32, :])


_CACHE = {}


def _build():
    if "nc" in _CACHE:
        return _CACHE["nc"]
    nc = bacc.Bacc("TRN2", target_bir_lowering=False, debug=False,
                   num_devices=N_CORES)
    io = {}

    def din(name, shape, dt=f32):
        io[name] = nc.dram_tensor(name, shape, dt, kind="ExternalInput").ap()

    din("edge16", [EDGE_DIM + XD, EP], f16)
    din("wblob", [128, WBW], f16)
    din("wme", [128, NT], f16)
    din("bias32", [128, 4])
    din("rows16", [1, 512], f16)
    din("a2h", [128, 128 * H], f16)
    io["pooled"] = nc.dram_tensor("pooled", [128, H], f32,
                                  kind="ExternalOutput").ap()

    with tile.TileContext(nc) as tc:
        _emit(nc, tc, io)
    nc.compile()
    _CACHE["nc"] = nc
    return nc


def _host_prep(inputs):
    x = np.asarray(inputs["x"], dtype=np.float32)
    ea = np.asarray(inputs["edge_attr"], dtype=np.float32)
    ei = np.asarray(inputs["edge_index"]).astype(np.int64)
    batch = np.asarray(inputs["batch"]).astype(np.int64)
    src, dst = ei[0], ei[1]
    gid = batch[dst]
    cnt = np.bincount(dst, minlength=N).astype(np.float32)
    w_all = 1.0 / np.maximum(cnt, 1.0)

    e4w = np.asarray(inputs["e4_w"], np.float32).reshape(128, 128, 128)
    a2h = np.ascontiguousarray(
        e4w.transpose(1, 0, 2).reshape(128, 128 * H).astype(np.float16))
    p2b = np.asarray(inputs["p2_b"], np.float32)
    b2 = np.einsum("i,kio->ko", p2b, e4w).astype(np.float16)   # [k, o]
    br = np.asarray(inputs["e4_b"], np.float32).reshape(128, 128)
    br2 = p2b @ br                                             # [o]

    wblob = np.zeros((128, WBW), np.float16)
    wblob[:, W_P2:W_P2 + 128] = np.asarray(inputs["p2_w"], np.float16)
    wblob[:, W_E2:W_E2 + 256] = np.asarray(inputs["e2_w"], np.float16)
    wblob[:, W_E30:W_E30 + 128] = np.asarray(inputs["e3_w"], np.float16)[0:128]
    wblob[:, W_E31:W_E31 + 128] = np.asarray(inputs["e3_w"], np.float16)[128:256]
    wblob[:, W_BR:W_BR + 128] = br.astype(np.float16)
    wblob[:, W_B2:W_B2 + 128] = b2
    wblob[0:NODE_DIM, W_P1:W_P1 + 128] = np.asarray(inputs["p1_w"], np.float16)
    wblob[NODE_DIM, W_P1:W_P1 + 128] = np.asarray(inputs["p1_b"], np.float16)
    wblob[0:EDGE_DIM, W_E1:W_E1 + 128] = np.asarray(inputs["e1_w"], np.float16)

    rows16 = np.zeros((1, 512), np.float16)
    rows16[0, :] = np.tile(np.asarray(inputs["e3_b"], np.float16), 4)

    bias_c = np.zeros((128, 4), np.float32)
    bias_c[:, B_E1] = np.asarray(inputs["e1_b"], np.float32)
    bias_c[:, B_E2:B_E2 + 2] = np.asarray(
        inputs["e2_b"], np.float32).reshape(2, 128).T

    com = {"wblob": wblob, "rows16": rows16, "bias32": bias_c, "a2h": a2h}
    com = {k: np.ascontiguousarray(v) for k, v in com.items()}

    in_maps = []
    wg_all = np.zeros((N_CORES, G_PER_CORE), np.float32)
    for c in range(N_CORES):
        ea_s = np.zeros((EP, EDGE_DIM), np.float32)
        xs_s = np.zeros((EP, XD), np.float32)
        w_s = np.zeros(EP, np.float32)
        for s in range(G_PER_CORE):
            es = np.where(gid == c * G_PER_CORE + s)[0]
            assert len(es) <= CAP, f"graph {c * G_PER_CORE + s}: {len(es)} edges"
            pos = s * CAP + np.arange(len(es))
            we = w_all[dst[es]]
            ea_s[pos] = ea[es]
            xs_s[pos, 0:NODE_DIM] = x[src[es]] * we[:, None]
            xs_s[pos, NODE_DIM] = we
            w_s[pos] = we
            wg_all[c, s] = we.sum()

        edge16 = np.zeros((EDGE_DIM + XD, EP), np.float16)
        edge16[0:EDGE_DIM] = ea_s.T
        edge16[EDGE_DIM:EDGE_DIM + XD] = xs_s.T

        m = dict(com)
        m["edge16"] = np.ascontiguousarray(edge16)
        m["wme"] = np.ascontiguousarray(w_s.reshape(NT, 128).T.astype(np.float16))
        in_maps.append(m)
    return in_maps, wg_all, br2


def _run(inputs, trace=False, tmpdir=None):
    nc = _build()
    in_maps, wg_all, br2 = _host_prep(inputs)
    if trace:
        # No egress in this sandbox: neutralize the artifact upload the
        # trace path performs after NTFF capture, and register the NTFF
        # hook module if the image's antenv package lacks axon_hooks.
        from concourse import bass_utils as _bu
        _bu.upload_artifacts = lambda d: d
        try:
            from antenv import axon_hooks  # noqa: F401
        except ImportError:
            import importlib.util, sys as _sys
            spec = importlib.util.spec_from_file_location(
                "antenv.axon_hooks", "/opt/trn_rl_repo/antenv/axon_hooks.py")
            mod = importlib.util.module_from_spec(spec)
            spec.loader.exec_module(mod)
            _sys.modules["antenv.axon_hooks"] = mod
    res = run_bass_kernel_spmd(nc, in_maps, list(range(N_CORES)),
                               trace=trace, tmpdir=tmpdir)
    out = np.empty((G, H), np.float32)
    for c in range(N_CORES):
        p = res.results[c]["pooled"]
        acc = np.zeros((G_PER_CORE, H), np.float32)
        for j in range(4):
            acc += p[32 * j:32 * j + G_PER_CORE]
            acc += p[32 * j + G_PER_CORE:32 * j + 2 * G_PER_CORE]
        acc += wg_all[c][:, None] * br2[None, :]   # W_g * (p2b @ Br)
        out[c * G_PER_CORE:(c + 1) * G_PER_CORE, :] = acc
    return out, res


def kernel(**inputs) -> np.ndarray:
    out, _ = _run(inputs)
    return out


# revision 32
# speedup vs baseline: 1.4738x; 1.0754x over previous
"""Trainium2 Bass kernel for nn_Net_16174846837292 (NNConv GNN message passing).

Strategy (graph-sharded, aggregation-folded, single fp16 a2 pass):
  pooled[g,o] = sum_{e: batch[dst[e]]=g} w_e * msg[e,o],  w_e = 1/max(cnt[dst_e],1)
  msg[e,o]    = sum_{k,i} e3[e,k]*h[src_e,i]*e4w[k,i*128+o] + sum_i h[src_e,i]*e4b[i*128+o]
  => pooled[g,o] = sum_k ZG_g[:,k]^T A2f[:,k*128+o] + HW_g^T Br
     ZG_g[i,k] = sum_{e in g} (w_e h[src_e,i]) e3[e,k],  HW_g[i] = sum_e w_e h[src_e,i]

Sharding: edges grouped by the graph of their destination node; 8 graphs per
core, so in-degree weights are per-edge host constants and NO collectives are
needed. Per-core edges pack into 8 slots of 192 (64-aligned segments).

Device pipeline (all PE operands 16-bit; fp32 would cost two array passes):
- host pre-gathers x[src_e], scales it by w_e and appends a w_e row, matched
  against [p1w; p1b]: the node MLP needs no bias handling and no per-tile
  w_e scaling on the device at all.
- last layer of each MLP runs edge-major (previous activations as the matmul
  stationary operand): no transposes, no gathers anywhere.
- ZG accumulates per graph slot in PSUM ([ZG | HW] via a ones column), then
  splits into bf16 hi/lo halves [zh|zl] (bf16 = fp32 exponent range, no
  subnormal trouble).
- the 4 MB fp16 a2 streams once as the moving operand against the 16-col
  [zh|zl] stationaries, col-tiled 4-wide across the PE array; the e4-bias
  term rides the same accumulation as hw_f @ Br.
- big a2 DMAs are gated behind the small input loads (a big transfer issued
  early monopolizes the 16 SDMA engines and stalls every input behind it),
  and tiny warmup matmuls keep the PE HAM clock at 2.4 GHz while they land.

Measured error vs the fp32 reference: ~4e-4 of output scale.
"""

import numpy as np
from contextlib import ExitStack

import concourse.bass as bass
import concourse.tile as tile
from concourse import bacc, mybir
from concourse.bass_utils import run_bass_kernel_spmd

N_CORES = 8
N, E, G, H = 4096, 8192, 64, 128
NODE_DIM, EDGE_DIM = 11, 5
G_PER_CORE = G // N_CORES          # 8 graph slots per core
CAP = 192                          # edge slots per graph (64-aligned segments)
EP = G_PER_CORE * CAP              # 1536 edge slots per core
NT = EP // 128                     # 12 edge tiles per core
NCH = EP // 512                    # 3 512-wide chunks for the feature-major MLPs
XD = NODE_DIM + 1                  # xs rows: w*x (11) + w (1)

f32 = mybir.dt.float32
f16 = mybir.dt.float16
bf16 = mybir.dt.bfloat16
AF = mybir.ActivationFunctionType
OP = mybir.AluOpType

# wblob column map (f16 weights packed into one [128, 1024] blob)
W_P2, W_E2, W_E30, W_E31, W_BR, W_P1, W_E1, W_B2 = (
    0, 128, 384, 512, 640, 768, 896, 1024)
WBW = 1152
# bias32 column map ([128, 4] f32)
B_E1, B_E2 = 0, 1


def _slot_segments(s):
    """(tile, p0, p1) segments of graph slot s in the (p, t) edge grid."""
    segs, a, end = [], s * CAP, (s + 1) * CAP
    while a < end:
        t, p0 = divmod(a, 128)
        take = min(128 - p0, end - a)
        segs.append((t, p0, p0 + take))
        a += take
    return segs


def _emit(nc, tc, io):
    es = ExitStack()
    const = es.enter_context(tc.tile_pool(name="const", bufs=1))
    a2pool = es.enter_context(tc.tile_pool(name="a2pool", bufs=1))
    big = es.enter_context(tc.tile_pool(name="big", bufs=1))
    work = es.enter_context(tc.tile_pool(name="work", bufs=4))
    psA = es.enter_context(tc.tile_pool(name="psA", bufs=3, space="PSUM"))
    psB = es.enter_context(tc.tile_pool(name="psB", bufs=1, space="PSUM"))
    psZ = es.enter_context(tc.tile_pool(name="psZ", bufs=3, space="PSUM"))
    psO = es.enter_context(tc.tile_pool(name="psO", bufs=1, space="PSUM"))

    with es:
        a2 = a2pool.tile([128, 128 * H], f16, tag="a2")

        wblob = const.tile([128, WBW], f16, tag="wblob")
        nc.sync.dma_start(wblob[:], io["wblob"][:, :])
        eaT = const.tile([EDGE_DIM, EP], f16, tag="eaT")
        nc.sync.dma_start(eaT[:], io["edge16"][0:EDGE_DIM, :])
        xsT = const.tile([XD, EP], f16, tag="xsT")
        last_sync = nc.sync.dma_start(xsT[:], io["edge16"][EDGE_DIM:EDGE_DIM + XD, :])
        wme = const.tile([128, NT], f16, tag="wme")
        nc.gpsimd.dma_start(wme[:], io["wme"][:, :])
        bias32 = const.tile([128, 4], f32, tag="bias32")
        nc.gpsimd.dma_start(bias32[:], io["bias32"][:, :])
        rows16 = const.tile([1, 512], f16, tag="rows16")
        last_dma = nc.gpsimd.dma_start(rows16[:], io["rows16"][:, :])

        # a2 halves on the idle sync/gpsimd queues, gated behind the last
        # small input load: a big transfer issued early monopolizes the 16
        # SDMA engines and delays every small input DMA behind it
        KSYNC = 72 * 128
        a2d0 = nc.sync.dma_start(a2[:, 0:KSYNC], io["a2h"][:, 0:KSYNC])
        a2d1 = nc.gpsimd.dma_start(a2[:, KSYNC:128 * H], io["a2h"][:, KSYNC:128 * H])
        for ad in (a2d0, a2d1):
            for dep in (last_dma, last_sync):
                bass._add_dep_helper(ad.ins, dep.ins, sync=True,
                                     reason="a2 after small input DMAs")

        # spin tiny matmuls while input DMAs land so HAM is warm (2.4 GHz)
        # when the real pipeline starts
        ones_r = const.tile([1, 128], f16, tag="ones_r")
        nc.vector.memset(ones_r[:], 1.0)
        for _ in range(40):
            pw = psA.tile([128, 512], f32, tag="mlp")
            nc.tensor.matmul(pw[0:16, 0:16], ones_r[:, 0:16], ones_r[:, 0:16],
                             start=True, stop=True)

        # ---- feature-major MLP interiors (epilogues split ACT/DVE) ----------
        relu1 = big.tile([128, EP], f16, tag="relu1")
        e1o = big.tile([128, EP], f16, tag="e1o")
        e2o0 = big.tile([128, EP], f16, tag="e2o0")
        e2o1 = big.tile([128, EP], f16, tag="e2o1")
        for q in range(NCH):
            sl = slice(q * 512, (q + 1) * 512)
            ps = psA.tile([128, 512], f32, tag="mlp")
            nc.tensor.matmul(ps[:], wblob[0:EDGE_DIM, W_E1:W_E1 + 128],
                             eaT[:, sl], start=True, stop=True)
            nc.scalar.activation(e1o[:, sl], ps[:], AF.Relu,
                                 bias=bias32[:, B_E1:B_E1 + 1])
            ps2 = psA.tile([128, 512], f32, tag="mlp")
            nc.tensor.matmul(ps2[:], wblob[0:XD, W_P1:W_P1 + 128],
                             xsT[:, sl], start=True, stop=True)
            nc.vector.tensor_scalar_max(relu1[:, sl], ps2[:], 0.0)
        for q in range(NCH):
            for m, e2o in enumerate((e2o0, e2o1)):
                sl = slice(q * 512, (q + 1) * 512)
                ps = psA.tile([128, 512], f32, tag="mlp")
                nc.tensor.matmul(ps[:], wblob[:, W_E2 + m * 128:W_E2 + (m + 1) * 128],
                                 e1o[:, sl], start=True, stop=True)
                if m == 0:
                    nc.scalar.activation(e2o[:, sl], ps[:], AF.Relu,
                                         bias=bias32[:, B_E2 + m:B_E2 + m + 1])
                else:
                    nc.vector.tensor_scalar(e2o[:, sl], ps[:],
                                            bias32[:, B_E2 + m:B_E2 + m + 1],
                                            0.0, op0=OP.add, op1=OP.max)

        # broadcast e3b (tiled 4x) to all partitions for 512-wide adds
        pbc2 = psA.tile([128, 512], f32, tag="mlp")
        nc.tensor.matmul(pbc2[:], ones_r[:], rows16[0:1, 0:512],
                         start=True, stop=True)
        e3bb = const.tile([128, 512], f32, tag="e3bb")
        nc.scalar.copy(e3bb[:], pbc2[:])

        # ---- edge-major last layers, 4-tile groups, 512-wide epilogues ------
        # h_big[e, t, i] = relu1[:, e].T @ p2w + p2b  (w_e, p1b folded on host)
        # e3x[e, t, k]   = relu(e2o[:, e].T @ e3w + e3b); col H = 1.0
        h_big = big.tile([128, NT, H], f16, tag="hbig")
        e3x = big.tile([128, NT, H + 1], f16, tag="e3x")
        nc.gpsimd.memset(e3x[:, :, H:H + 1], 1.0)
        for g4 in range(NT // 4):
            psh = psB.tile([128, 512], f32, tag="he4")
            pse = psA.tile([128, 512], f32, tag="mlp")
            for j in range(4):
                t = g4 * 4 + j
                sl = slice(t * 128, (t + 1) * 128)
                jj = slice(j * 128, (j + 1) * 128)
                nc.tensor.matmul(psh[:, jj], relu1[:, sl], wblob[:, W_P2:W_P2 + 128],
                                 start=True, stop=True)
                nc.tensor.matmul(pse[:, jj], e2o0[:, sl], wblob[:, W_E30:W_E30 + 128],
                                 start=True, stop=False)
                nc.tensor.matmul(pse[:, jj], e2o1[:, sl], wblob[:, W_E31:W_E31 + 128],
                                 start=False, stop=True)
            g4s = slice(4 * g4, 4 * g4 + 4)
            if g4 % 2 == 0:
                nc.scalar.copy(h_big[:, g4s, :], psh[:])
            else:
                nc.vector.tensor_copy(h_big[:, g4s, :], psh[:])
            t4 = work.tile([128, 512], f32, tag="t4")
            nc.vector.tensor_tensor(t4[:], pse[:], e3bb[:], op=OP.add)
            nc.scalar.activation(e3x[:, g4s, 0:H], t4[:], AF.Relu)

        # ---- per-graph ZG accumulation + bf16 hi/lo split --------------------
        # pz cols 0:128 = ZG (stationary h), col 128 = HW (ones col of e3x)
        zg2 = big.tile([128, 2 * G_PER_CORE, H], bf16, tag="zg2")
        # interleaved [hw_0 s_0 hw_1 s_1 ...]; strided stationary APs below
        hws = work.tile([128, 2 * G_PER_CORE], f16, tag="hws")
        for s in range(G_PER_CORE):
            segs = _slot_segments(s)
            pz = psZ.tile([128, H + 2], f32, tag="zg")
            # the S chain starts only after the ZG chain closes: a matmul
            # with start=True clears has_written for its partitions across
            # the whole PSUM region, killing any open chain there
            for n, (t, p0, p1) in enumerate(segs):
                nc.tensor.matmul(pz[:, 0:H + 1], h_big[p0:p1, t, :],
                                 e3x[p0:p1, t, :],
                                 start=(n == 0), stop=(n == len(segs) - 1))
            for n, (t, p0, p1) in enumerate(segs):
                nc.tensor.matmul(pz[:, H + 1:H + 2], e3x[p0:p1, t, 0:H],
                                 wme[p0:p1, t:t + 1],
                                 start=(n == 0), stop=(n == len(segs) - 1))
            if s % 2 == 0:
                nc.scalar.copy(zg2[:, s, :], pz[:, 0:H])
            else:
                nc.vector.tensor_copy(zg2[:, s, :], pz[:, 0:H])
            nc.vector.tensor_tensor(zg2[:, G_PER_CORE + s, :], pz[:, 0:H],
                                    zg2[:, s, :], op=OP.subtract)
            nc.vector.tensor_copy(hws[:, 2 * s:2 * s + 2], pz[:, H:H + 2])

        # ---- final a2 contraction: a2 streams as the moving operand ----------
        # the e4-bias term (hw_f @ Br) rides as the last accumulation step of
        # col-group 0 (hw_f cols 8:16 are zeros)
        ot = work.tile([128, 128], f32, tag="ot")
        nc.gpsimd.memset(ot[:], 0.0)
        po = psO.tile([128, 128], f32, tag="out")
        # groups 1-3 take 30 k's each, group 0 takes 38 + the bias terms:
        # early groups free their output rows while group 0 still streams
        KL = [list(range(0, 8)) + list(range(98, 128)),
              list(range(8, 38)), list(range(38, 68)), list(range(68, 98))]
        for r in range(38):
            for j in range(4):
                if r >= len(KL[j]):
                    continue
                k = KL[j][r]
                nc.tensor.matmul(po[32 * j:32 * j + 16, :], zg2[:, :, k],
                                 a2[:, k * 128:(k + 1) * 128],
                                 start=(r == 0),
                                 stop=(j != 0 and r == len(KL[j]) - 1),
                                 tile_position=(0, 32 * j))
        nc.tensor.matmul(po[0:G_PER_CORE, :], hws[:, 0:16:2],
                         wblob[:, W_BR:W_BR + 128],
                         start=False, stop=False, tile_position=(0, 0))
        nc.tensor.matmul(po[0:G_PER_CORE, :], hws[:, 1:16:2],
                         wblob[:, W_B2:W_B2 + 128],
                         start=False, stop=True, tile_position=(0, 0))
        for j in (1, 2, 3, 0):
            if j % 2 == 0:
                nc.scalar.copy(ot[32 * j:32 * j + 16, :], po[32 * j:32 * j + 16, :])
            else:
                nc.vector.tensor_copy(ot[32 * j:32 * j + 16, :],
                                      po[32 * j:32 * j + 16, :])
        nc.sync.dma_start(io["pooled"][32:128, :], ot[32:128, :])
        nc.sync.dma_start(io["pooled"][0:32, :], ot[0:32, :])


_CACHE = {}


def _build():
    if "nc" in _CACHE:
        return _CACHE["nc"]
    nc = bacc.Bacc("TRN2", target_bir_lowering=False, debug=False,
                   num_devices=N_CORES)
    io = {}

    def din(name, shape, dt=f32):
        io[name] = nc.dram_tensor(name, shape, dt, kind="ExternalInput").ap()

    din("edge16", [EDGE_DIM + XD, EP], f16)
    din("wblob", [128, WBW], f16)
    din("wme", [128, NT], f16)
    din("bias32", [128, 4])
    din("rows16", [1, 512], f16)
    din("a2h", [128, 128 * H], f16)
    io["pooled"] = nc.dram_tensor("pooled", [128, H], f32,
                                  kind="ExternalOutput").ap()

    with tile.TileContext(nc) as tc:
        _emit(nc, tc, io)
    nc.compile()
    _CACHE["nc"] = nc
    return nc


def _host_prep(inputs):
    x = np.asarray(inputs["x"], dtype=np.float32)
    ea = np.asarray(inputs["edge_attr"], dtype=np.float32)
    ei = np.asarray(inputs["edge_index"]).astype(np.int64)
    batch = np.asarray(inputs["batch"]).astype(np.int64)
    src, dst = ei[0], ei[1]
    gid = batch[dst]
    cnt = np.bincount(dst, minlength=N).astype(np.float32)
    w_all = 1.0 / np.maximum(cnt, 1.0)

    e4w = np.asarray(inputs["e4_w"], np.float32).reshape(128, 128, 128)
    a2h = np.ascontiguousarray(
        e4w.transpose(1, 0, 2).reshape(128, 128 * H).astype(np.float16))
    p2b = np.asarray(inputs["p2_b"], np.float32)
    b2 = np.einsum("i,kio->ko", p2b, e4w).astype(np.float16)   # [k, o]
    br = np.asarray(inputs["e4_b"], np.float32).reshape(128, 128)
    br2 = p2b @ br                                             # [o]

    wblob = np.zeros((128, WBW), np.float16)
    wblob[:, W_P2:W_P2 + 128] = np.asarray(inputs["p2_w"], np.float16)
    wblob[:, W_E2:W_E2 + 256] = np.asarray(inputs["e2_w"], np.float16)
    wblob[:, W_E30:W_E30 + 128] = np.asarray(inputs["e3_w"], np.float16)[0:128]
    wblob[:, W_E31:W_E31 + 128] = np.asarray(inputs["e3_w"], np.float16)[128:256]
    wblob[:, W_BR:W_BR + 128] = br.astype(np.float16)
    wblob[:, W_B2:W_B2 + 128] = b2
    wblob[0:NODE_DIM, W_P1:W_P1 + 128] = np.asarray(inputs["p1_w"], np.float16)
    wblob[NODE_DIM, W_P1:W_P1 + 128] = np.asarray(inputs["p1_b"], np.float16)
    wblob[0:EDGE_DIM, W_E1:W_E1 + 128] = np.asarray(inputs["e1_w"], np.float16)

    rows16 = np.zeros((1, 512), np.float16)
    rows16[0, :] = np.tile(np.asarray(inputs["e3_b"], np.float16), 4)

    bias_c = np.zeros((128, 4), np.float32)
    bias_c[:, B_E1] = np.asarray(inputs["e1_b"], np.float32)
    bias_c[:, B_E2:B_E2 + 2] = np.asarray(
        inputs["e2_b"], np.float32).reshape(2, 128).T

    com = {"wblob": wblob, "rows16": rows16, "bias32": bias_c, "a2h": a2h}
    com = {k: np.ascontiguousarray(v) for k, v in com.items()}

    in_maps = []
    wg_all = np.zeros((N_CORES, G_PER_CORE), np.float32)
    for c in range(N_CORES):
        ea_s = np.zeros((EP, EDGE_DIM), np.float32)
        xs_s = np.zeros((EP, XD), np.float32)
        w_s = np.zeros(EP, np.float32)
        for s in range(G_PER_CORE):
            es = np.where(gid == c * G_PER_CORE + s)[0]
            assert len(es) <= CAP, f"graph {c * G_PER_CORE + s}: {len(es)} edges"
            pos = s * CAP + np.arange(len(es))
            we = w_all[dst[es]]
            ea_s[pos] = ea[es]
            xs_s[pos, 0:NODE_DIM] = x[src[es]] * we[:, None]
            xs_s[pos, NODE_DIM] = we
            w_s[pos] = we
            wg_all[c, s] = we.sum()

        edge16 = np.zeros((EDGE_DIM + XD, EP), np.float16)
        edge16[0:EDGE_DIM] = ea_s.T
        edge16[EDGE_DIM:EDGE_DIM + XD] = xs_s.T

        m = dict(com)
        m["edge16"] = np.ascontiguousarray(edge16)
        m["wme"] = np.ascontiguousarray(w_s.reshape(NT, 128).T.astype(np.float16))
        in_maps.append(m)
    return in_maps, wg_all, br2


def _run(inputs, trace=False, tmpdir=None):
    nc = _build()
    in_maps, wg_all, br2 = _host_prep(inputs)
    if trace:
        # No egress in this sandbox: neutralize the artifact upload the
        # trace path performs after NTFF capture, and register the NTFF
        # hook module if the image's antenv package lacks axon_hooks.
        from concourse import bass_utils as _bu
        _bu.upload_artifacts = lambda d: d
        try:
            from antenv import axon_hooks  # noqa: F401
        except ImportError:
            import importlib.util, sys as _sys
            spec = importlib.util.spec_from_file_location(
                "antenv.axon_hooks", "/opt/trn_rl_repo/antenv/axon_hooks.py")
            mod = importlib.util.module_from_spec(spec)
            spec.loader.exec_module(mod)
            _sys.modules["antenv.axon_hooks"] = mod
    res = run_bass_kernel_spmd(nc, in_maps, list(range(N_CORES)),
                               trace=trace, tmpdir=tmpdir)
    out = np.empty((G, H), np.float32)
    for c in range(N_CORES):
        p = res.results[c]["pooled"]
        acc = np.zeros((G_PER_CORE, H), np.float32)
        for j in range(4):
            acc += p[32 * j:32 * j + G_PER_CORE]
            acc += p[32 * j + G_PER_CORE:32 * j + 2 * G_PER_CORE]
        acc += wg_all[c][:, None] * br2[None, :]   # W_g * (p2b @ Br)
        out[c * G_PER_CORE:(c + 1) * G_PER_CORE, :] = acc
    return out, res


def kernel(**inputs) -> np.ndarray:
    out, _ = _run(inputs)
    return out
